# revision 3
# baseline (speedup 1.0000x reference)
"""Self-contained Trainium2 Bass kernel for 16-head cross-attention MHA.

Problem: B=2, SQ=SK=2048, D=1024, H=16, key_size=64 (fp32 in/out).

Sharding (8 cores): data-parallel over batch (2) x tensor-parallel over
head groups (4 heads per core). Each core computes its 4 heads'
Q/K/V projections (column slices of wq/wk/wv), attention, and a partial
output projection (row slice of wo). Host sums the 4 partial outputs per
batch and adds the (bv @ wo + bo) correction (probs sum to 1, so bv
contributes exactly bv @ wo; bk cancels in softmax).

Device pipeline per core (bf16 matmuls, fp32 PSUM accumulation). The
kernel is ScalarE-paced: exp over the 4 x 2048 x 2048 score matrix is
~140us of ACT time, so everything else is structured to hide under it
within the 8-bank PSUM budget (tags: "ss" 2x[128,1024], "cc" 4x[128,512]):

  1. Prefix: K^T/Q^T projections for head-pair 0 only (nt0), so the
     first score matmul issues ~25us in. x^T arrives pre-transposed in
     bf16 from the host; xdT streams through a 3-chunk pool.
  2. Attention runs as 4 phases (head-pair, m-half). Scores^T tiles
     ([key_pos, query] layout, contraction = head_dim on partitions)
     are exp'd by ScalarE on [128,1024] PSUM tiles (scale=1/8 fused,
     no max-subtraction: scores ~ N(0,1), exp is safe) into bf16 SBUF.
  3. ctx for phase i is PHASE-SHIFTED into phase i+1: its 4 PSUM
     accumulation chains (one per m-quarter; V' carries a ones column so
     row 64 accumulates the softmax denominator Z) consume phase i's exp
     tiles while phase i+1's scores stream. This frees the "cc" banks
     during phase 0 to absorb the V projection and the nt1 K/Q
     projections, dribbled into phase 0's key-tile loop. The last phase
     runs its predecessor's ctx at 2x rate in its front half and its own
     ctx in its back half, leaving only norm + out-proj for the tail.
  4. Normalization: U' is evicted early to SBUF (frees the PSUM bank),
     Z row is partition-broadcast via GpSimd, single-pass DVE reciprocal,
     DVE multiply into ctx^T bf16.
  5. Output projection from ctx^T tiles against wo rows in the tail;
     PSUM evictions on the (by then idle) ScalarE; partial out to HBM.

Measured on 8 axon-tunneled trn2 cores: ~237us HW exec, rel err 4.6e-3
(all-bf16 data path; error is bf16 input-cast dominated).
"""

import os
import sys

for _p in ("/opt/trn_rl_repo", "/root/.axon_site/_ro/trn_rl_repo"):
    if os.path.isdir(_p) and _p not in sys.path:
        sys.path.insert(0, _p)

import numpy as np
import ml_dtypes

BF16 = ml_dtypes.bfloat16

B = 2
S = 2048          # SQ == SK
D = 1024
H = 16
KEY = 64
HPC = 4           # heads per core
NPC = HPC * KEY   # 256 per-core slice of D
KT = D // 128     # 8 contraction tiles for projections
NT = NPC // 128   # 2 head-pair tiles
MC = S // 512     # 4 m-chunks of 512
JT = S // 128     # 16 key tiles

_NC = None
LAST_RESULTS = None  # BassKernelResults of the most recent run (for test.py)


def _build_nc():
    import concourse.tile as tile
    from concourse import bacc, mybir

    FP32 = mybir.dt.float32
    BF = mybir.dt.bfloat16
    AF = mybir.ActivationFunctionType

    nc = bacc.Bacc("TRN2", target_bir_lowering=False, debug=False, num_devices=8)

    xdT = nc.dram_tensor("xdT", [D, S], BF, kind="ExternalInput").ap()
    xeT = nc.dram_tensor("xeT", [D, S], BF, kind="ExternalInput").ap()
    wq_d = nc.dram_tensor("wq", [D, NPC], BF, kind="ExternalInput").ap()
    wk_d = nc.dram_tensor("wk", [D, NPC], BF, kind="ExternalInput").ap()
    wv_d = nc.dram_tensor("wv", [D, NPC], BF, kind="ExternalInput").ap()
    wo_d = nc.dram_tensor("wo", [NPC, D], BF, kind="ExternalInput").ap()
    bq_d = nc.dram_tensor("bq", [NT, 128, 1], FP32, kind="ExternalInput").ap()
    o_d = nc.dram_tensor("o", [S, D], FP32, kind="ExternalOutput").ap()

    with tile.TileContext(nc) as tc:
        with (
            tc.tile_pool(name="consts", bufs=1) as consts,
            tc.tile_pool(name="acts", bufs=1) as acts,
            tc.tile_pool(name="zp", bufs=2) as zp,
            tc.tile_pool(name="up", bufs=6) as up,
            tc.tile_pool(name="zbp", bufs=2) as zbp,
            tc.tile_pool(name="osb", bufs=4) as osb,
        ):
            # ---- resident weights ----
            wq_sb = consts.tile([128, KT, NPC], BF, tag="wq")
            nc.sync.dma_start(wq_sb[:], wq_d.rearrange("(t p) n -> p t n", p=128))
            wk_sb = consts.tile([128, KT, NPC], BF, tag="wk")
            nc.sync.dma_start(wk_sb[:], wk_d.rearrange("(t p) n -> p t n", p=128))
            wv_sb = consts.tile([128, KT, NPC], BF, tag="wv")
            nc.sync.dma_start(wv_sb[:], wv_d.rearrange("(t p) n -> p t n", p=128))
            wo_sb = consts.tile([128, NT, D], BF, tag="wo")
            nc.sync.dma_start(wo_sb[:], wo_d.rearrange("(t p) n -> p t n", p=128))
            bq_sb = consts.tile([128, NT, 1], FP32, tag="bq")
            nc.sync.dma_start(bq_sb[:], bq_d.rearrange("t p o -> p t o"))

            # ---- activations kept resident ----
            QT_sb = acts.tile([128, NT, S], BF, tag="QT")    # [head_dim, m]
            KT_sb = acts.tile([128, NT, S], BF, tag="KT")    # [head_dim, j]
            v_sb = acts.tile([128, JT, HPC, KEY + 1], BF, tag="v")  # V' + ones col
            ctxT_sb = acts.tile([128, NT, S], BF, tag="ctxT")

            nc.vector.memset(v_sb[:, :, :, KEY:KEY + 1], 1.0)

            # ================= single PSUM pool =================
            # "ss": 2x[128,1024] (4 banks) scores / out-proj
            # "cc": 4x[128,512] (4 banks) proj chains, V chains, ctx chains
            # ctx for phase i is PHASE-SHIFTED: its MMs run during phase i+1,
            # so cc is free during phase 0 to absorb V / K-nt1 / Q-nt1.
            with (
                tc.tile_pool(name="expp", bufs=34) as expp,
                tc.tile_pool(name="xep", bufs=1) as xep,
                tc.tile_pool(name="xdp", bufs=3) as xdp,
                tc.tile_pool(name="ps", bufs=2, space="PSUM") as ps,
                tc.tile_pool(name="cp", bufs=4, space="PSUM") as cp,
            ):
                xeT_sb = xep.tile([128, KT, S], BF, tag="xeT")
                for kt in range(KT):
                    nc.sync.dma_start(
                        xeT_sb[:, kt, :],
                        xeT.rearrange("(t p) m -> p t m", p=128)[:, kt, :],
                    )
                xdT_r = xdT.rearrange("(t p) m -> p t m", p=128)

                def q_chunk(kt, eng=None):
                    t = xdp.tile([128, S], BF, tag="xd", name=f"xdc_{kt}")
                    nc.sync.dma_start(t[:], xdT_r[:, kt, :])
                    return t

                def proj_pass(w_sb, nt, chains, x_tiles, kt):
                    for mc in range(MC):
                        nc.tensor.matmul(
                            chains[mc],
                            w_sb[:, kt, nt * 128:(nt + 1) * 128],
                            x_tiles[mc // 4] if isinstance(x_tiles, list)
                            else x_tiles[:, kt, mc * 512:(mc + 1) * 512],
                            start=(kt == 0),
                            stop=(kt == KT - 1),
                        )

                def evict_proj(chains, dst, nt, bias):
                    for mc in range(MC):
                        out_ap = dst[:, nt, mc * 512:(mc + 1) * 512]
                        if bias is not None:
                            nc.vector.tensor_scalar_add(out_ap, chains[mc], bias[:, nt, :])
                        else:
                            nc.vector.tensor_copy(out_ap, chains[mc])

                # ---- prefix: K-nt0 then Q-nt0 (heads 0,1) ----
                k0 = [cp.tile([128, 512], FP32, tag="cc", name=f"k0_{i}") for i in range(4)]
                for kt in range(KT):
                    for mc in range(MC):
                        nc.tensor.matmul(
                            k0[mc][:], wk_sb[:, kt, 0:128],
                            xeT_sb[:, kt, mc * 512:(mc + 1) * 512],
                            start=(kt == 0), stop=(kt == KT - 1),
                        )
                evict_proj([c[:] for c in k0], KT_sb, 0, None)
                q0 = [cp.tile([128, 512], FP32, tag="cc", name=f"q0_{i}") for i in range(4)]
                for kt in range(KT):
                    xc = q_chunk(kt)
                    for mc in range(MC):
                        nc.tensor.matmul(
                            q0[mc][:], wq_sb[:, kt, 0:128],
                            xc[:, mc * 512:(mc + 1) * 512],
                            start=(kt == 0), stop=(kt == KT - 1),
                        )
                evict_proj([c[:] for c in q0], QT_sb, 0, bq_sb)

                # ---- phases: scores(si) + shifted ctx(si-1) + dribbles ----
                order = [(0, 0), (0, 1), (1, 0), (1, 1)]
                rows = [0, KEY]
                prev = None  # (hp, mh, exp_tiles)
                drib = {}    # state for phase-0 dribbles

                def emit_ctx_step(hp, mh, jt, exp_row, ccs):
                    for hh in range(2):
                        h = hp * 2 + hh
                        for q in range(2):
                            nc.tensor.matmul(
                                ccs[hh * 2 + q][0:KEY + 1, :],
                                v_sb[:, jt, h, :],
                                exp_row[hh][:, q * 512:(q + 1) * 512],
                                start=(jt == 0),
                                stop=(jt == JT - 1),
                            )

                def emit_norm(hp, mh, ccs):
                    m0 = mh * 1024
                    for hh in range(2):
                        row = rows[hh]
                        for q in range(2):
                            c = ccs[hh * 2 + q]
                            u = up.tile([KEY + 1, 512], FP32, tag="u")
                            nc.vector.tensor_copy(u[:], c[0:KEY + 1, :])
                            zraw = zp.tile([1, 512], FP32, tag="z")
                            nc.vector.tensor_copy(zraw[:], u[KEY:KEY + 1, :])
                            zb = zbp.tile([KEY, 512], FP32, tag="zb")
                            nc.gpsimd.partition_broadcast(zb[:], zraw[:])
                            zbr = zbp.tile([KEY, 512], FP32, tag="zbr")
                            nc.vector.reciprocal_approx_fast(zbr[:], zb[:])
                            nc.vector.tensor_mul(
                                ctxT_sb[row:row + KEY, hp, m0 + q * 512:m0 + (q + 1) * 512],
                                u[0:KEY, :],
                                zbr[:],
                            )

                for si, (hp, mh) in enumerate(order):
                    m0 = mh * 1024
                    last = si == len(order) - 1
                    cur_cc = None
                    prev_cc = None
                    if prev is not None:
                        prev_cc = [cp.tile([128, 512], FP32, tag="cc", name=f"cc_{si}_{i}")
                                   for i in range(4)]
                    cur_exps = []
                    for jt in range(JT):
                        exp_row = []
                        # Packed scores: the two K=64 head-halves are issued
                        # back-to-back per q-chunk so their auto-derived PE
                        # row-groups (0,0)/(64,0) stream concurrently.
                        sst = [ps.tile([128, 1024], FP32, tag="ss",
                                       name=f"ss_{si}_{jt}_{hh2}") for hh2 in range(2)]
                        for q in range(2):
                            for hh in range(2):
                                row = rows[hh]
                                nc.tensor.matmul(
                                    sst[hh][:, q * 512:(q + 1) * 512],
                                    KT_sb[row:row + KEY, hp, jt * 128:(jt + 1) * 128],
                                    QT_sb[row:row + KEY, hp, m0 + q * 512:m0 + (q + 1) * 512],
                                    start=True, stop=True,
                                )
                        for hh in range(2):
                            et = expp.tile([128, 1024], BF, tag="exp")
                            nc.scalar.activation(et[:], sst[hh][:], AF.Exp, scale=0.125)
                            exp_row.append(et)
                        cur_exps.append(exp_row)
                        if prev is not None and not last:
                            emit_ctx_step(prev[0], prev[1], jt, prev[2][jt], prev_cc)
                        if last:
                            # front half: finish prev phase's ctx at 2x rate;
                            # back half: this phase's own ctx at 2x rate (its
                            # exp tiles exist by then) -> nothing left for the
                            # tail but normalization + output projection.
                            if jt < 8:
                                for j2 in (jt * 2, jt * 2 + 1):
                                    emit_ctx_step(prev[0], prev[1], j2, prev[2][j2], prev_cc)
                                if jt == 7:
                                    emit_norm(prev[0], prev[1], prev_cc)
                            else:
                                if jt == 8:
                                    cur_cc = [cp.tile([128, 512], FP32, tag="cc",
                                                      name=f"cc_last_{i}") for i in range(4)]
                                for j2 in ((jt - 8) * 2, (jt - 8) * 2 + 1):
                                    emit_ctx_step(hp, mh, j2, cur_exps[j2], cur_cc)
                        if si == 0:
                            # jt 0-7: V pairs (ctx of phase 0 needs them in
                            # phase 1); jt 8-11: K-nt1; jt 12-15: Q-nt1
                            # (nt1 first needed by phase 2's scores).
                            if jt < 8:
                                jt0 = jt * 2
                                pv = [cp.tile([128, 512], FP32, tag="cc",
                                              name=f"pv_{jt0}_{d}") for d in range(2)]
                                for kt in range(KT):
                                    for d in range(2):
                                        nc.tensor.matmul(
                                            pv[d][:, 0:NPC],
                                            xeT_sb[:, kt, (jt0 + d) * 128:(jt0 + d + 1) * 128],
                                            wv_sb[:, kt, :],
                                            start=(kt == 0), stop=(kt == KT - 1),
                                        )
                                for d in range(2):
                                    nc.vector.tensor_copy(
                                        v_sb[:, jt0 + d, :, 0:KEY],
                                        pv[d][:, 0:NPC].rearrange("p (h k) -> p h k", h=HPC),
                                    )
                            elif jt < 12:
                                if jt == 8:
                                    drib["k1"] = [cp.tile([128, 512], FP32, tag="cc",
                                                          name=f"k1_{i}") for i in range(4)]
                                for kk in range(2):
                                    kt = (jt - 8) * 2 + kk
                                    for mc in range(MC):
                                        nc.tensor.matmul(
                                            drib["k1"][mc][:], wk_sb[:, kt, 128:256],
                                            xeT_sb[:, kt, mc * 512:(mc + 1) * 512],
                                            start=(kt == 0), stop=(kt == KT - 1),
                                        )
                                if jt == 11:
                                    evict_proj([c[:] for c in drib["k1"]], KT_sb, 1, None)
                            else:
                                if jt == 12:
                                    drib["q1"] = [cp.tile([128, 512], FP32, tag="cc",
                                                          name=f"q1_{i}") for i in range(4)]
                                for kk in range(2):
                                    kt = (jt - 12) * 2 + kk
                                    xc = q_chunk(kt)
                                    for mc in range(MC):
                                        nc.tensor.matmul(
                                            drib["q1"][mc][:], wq_sb[:, kt, 128:256],
                                            xc[:, mc * 512:(mc + 1) * 512],
                                            start=(kt == 0), stop=(kt == KT - 1),
                                        )
                                if jt == 15:
                                    evict_proj([c[:] for c in drib["q1"]], QT_sb, 1, bq_sb)
                    if last:
                        emit_norm(hp, mh, cur_cc)
                    elif prev is not None:
                        emit_norm(prev[0], prev[1], prev_cc)
                    prev = (hp, mh, cur_exps)

                # ================= output projection =================
                for mt in range(S // 128):
                    ot = osb.tile([128, D], FP32, tag="ot")
                    po = ps.tile([128, 1024], FP32, tag="ss", name=f"po_{mt}")
                    for dt in range(NT):
                        for ec in range(2):
                            nc.tensor.matmul(
                                po[:, ec * 512:(ec + 1) * 512],
                                ctxT_sb[:, dt, mt * 128:(mt + 1) * 128],
                                wo_sb[:, dt, ec * 512:(ec + 1) * 512],
                                start=(dt == 0),
                                stop=(dt == NT - 1),
                            )
                    nc.scalar.copy(ot[:], po[:])
                    nc.sync.dma_start(o_d[mt * 128:(mt + 1) * 128, :], ot[:])

    nc.compile()
    return nc


def _get_nc():
    global _NC
    if _NC is None:
        _NC = _build_nc()
    return _NC


def _maybe_register_ntff_hook():
    """Optional: register the axon NTFF profile hook so BASS_TRACE=1 yields
    HW exec times. No-op if unavailable (e.g. the grading environment)."""
    if "antenv.axon_hooks" in sys.modules:
        return
    try:
        import types

        if "/root/.axon_site" not in sys.path and os.path.isdir("/root/.axon_site"):
            sys.path.append("/root/.axon_site")
        from trn_agent_boot.trn_boot import _ntff_profile_via_ctypes

        hook = _ntff_profile_via_ctypes("/opt/axon/libaxon_pjrt.so")
        mod = types.ModuleType("antenv.axon_hooks")
        mod.get_axon_ntff_profile_hook = lambda: hook
        mod.set_axon_ntff_profile_hook = lambda h: None
        sys.modules["antenv.axon_hooks"] = mod
    except Exception:
        pass


def kernel(decoder_output, encoder_output, wq, bq, wk, bk, wv, bv, wo, bo):
    from concourse.bass_utils import run_bass_kernel_spmd

    global LAST_RESULTS

    decoder_output = np.asarray(decoder_output, dtype=np.float32)
    encoder_output = np.asarray(encoder_output, dtype=np.float32)
    wq = np.asarray(wq, dtype=np.float32)
    wk = np.asarray(wk, dtype=np.float32)
    wv = np.asarray(wv, dtype=np.float32)
    wo = np.asarray(wo, dtype=np.float32)
    bq = np.asarray(bq, dtype=np.float32)
    bv = np.asarray(bv, dtype=np.float32)
    bo = np.asarray(bo, dtype=np.float32)
    # bk is softmax-invariant (adds a per-query constant to every logit).

    if os.environ.get("BASS_TRACE"):
        _maybe_register_ntff_hook()

    nc = _get_nc()

    xT = {}
    for b in range(B):
        xT[("d", b)] = np.ascontiguousarray(decoder_output[b].T).astype(BF16)
        xT[("e", b)] = np.ascontiguousarray(encoder_output[b].T).astype(BF16)

    in_maps = []
    for c in range(8):
        b, hg = c // 4, c % 4
        sl = slice(hg * NPC, (hg + 1) * NPC)
        in_maps.append({
            "xdT": xT[("d", b)],
            "xeT": xT[("e", b)],
            "wq": wq[:, sl].astype(BF16),
            "wk": wk[:, sl].astype(BF16),
            "wv": wv[:, sl].astype(BF16),
            "wo": np.ascontiguousarray(wo[sl, :]).astype(BF16),
            "bq": bq[sl].reshape(NT, 128, 1),
        })

    res = run_bass_kernel_spmd(nc, in_maps, core_ids=list(range(8)))
    LAST_RESULTS = res

    correction = (bv @ wo + bo).astype(np.float32)  # probs sum to 1
    out = np.zeros((B, S, D), dtype=np.float32)
    for c in range(8):
        out[c // 4] += res.results[c]["o"]
    out += correction[None, None, :]
    return out



# revision 8
# speedup vs baseline: 1.0069x; 1.0069x over previous
"""Self-contained Trainium2 Bass kernel for 16-head cross-attention MHA.

Problem: B=2, SQ=SK=2048, D=1024, H=16, key_size=64 (fp32 in/out).

Sharding (8 cores): data-parallel over batch (2) x tensor-parallel over
head groups (4 heads per core). Each core computes its 4 heads'
Q/K/V projections (column slices of wq/wk/wv), attention, and a partial
output projection (row slice of wo). Host sums the 4 partial outputs per
batch and adds the (bv @ wo + bo) correction (probs sum to 1, so bv
contributes exactly bv @ wo; bk cancels in softmax).

Device pipeline per core (bf16 matmuls, fp32 PSUM accumulation). The
kernel is ScalarE-paced: exp over the 4 x 2048 x 2048 score matrix is
~140us of ACT time, so everything else is structured to hide under it
within the 8-bank PSUM budget (tags: "ss" 2x[128,1024], "cc" 4x[128,512]):

  1. Prefix: K^T/Q^T projections for head-pair 0 only (nt0), so the
     first score matmul issues ~25us in. x^T arrives pre-transposed in
     bf16 from the host; xdT streams through a 3-chunk pool.
  2. Attention runs as 4 phases (head-pair, m-half). Scores^T tiles
     ([key_pos, query] layout, contraction = head_dim on partitions)
     are exp'd by ScalarE on [128,1024] PSUM tiles (scale=1/8 fused,
     no max-subtraction: scores ~ N(0,1), exp is safe) into bf16 SBUF.
  3. ctx for phase i is PHASE-SHIFTED into phase i+1: its 4 PSUM
     accumulation chains (one per m-quarter; V' carries a ones column so
     row 64 accumulates the softmax denominator Z) consume phase i's exp
     tiles while phase i+1's scores stream. This frees the "cc" banks
     during phase 0 to absorb the V projection and the nt1 K/Q
     projections, dribbled into phase 0's key-tile loop. The last phase
     runs its predecessor's ctx at 2x rate in its front half and its own
     ctx in its back half, leaving only norm + out-proj for the tail.
  4. Normalization: U' is evicted early to SBUF (frees the PSUM bank),
     Z row is partition-broadcast via GpSimd, single-pass DVE reciprocal,
     DVE multiply into ctx^T bf16.
  5. Output projection from ctx^T tiles against wo rows in the tail;
     PSUM evictions on the (by then idle) ScalarE; partial out to HBM.

Measured on 8 axon-tunneled trn2 cores: ~237us HW exec, rel err 4.6e-3
(all-bf16 data path; error is bf16 input-cast dominated).
"""

import os
import sys

for _p in ("/opt/trn_rl_repo", "/root/.axon_site/_ro/trn_rl_repo"):
    if os.path.isdir(_p) and _p not in sys.path:
        sys.path.insert(0, _p)

import numpy as np
import ml_dtypes

BF16 = ml_dtypes.bfloat16

B = 2
S = 2048          # SQ == SK
D = 1024
H = 16
KEY = 64
HPC = 4           # heads per core
NPC = HPC * KEY   # 256 per-core slice of D
KT = D // 128     # 8 contraction tiles for projections
NT = NPC // 128   # 2 head-pair tiles
MC = S // 512     # 4 m-chunks of 512
JT = S // 128     # 16 key tiles

_NC = None
LAST_RESULTS = None  # BassKernelResults of the most recent run (for test.py)


def _build_nc():
    import concourse.tile as tile
    from concourse import bacc, mybir

    FP32 = mybir.dt.float32
    BF = mybir.dt.bfloat16
    AF = mybir.ActivationFunctionType

    nc = bacc.Bacc("TRN2", target_bir_lowering=False, debug=False, num_devices=8)

    xdT = nc.dram_tensor("xdT", [D, S], BF, kind="ExternalInput").ap()
    xeT = nc.dram_tensor("xeT", [D, S], BF, kind="ExternalInput").ap()
    wq_d = nc.dram_tensor("wq", [D, NPC], BF, kind="ExternalInput").ap()
    wk_d = nc.dram_tensor("wk", [D, NPC], BF, kind="ExternalInput").ap()
    wv_d = nc.dram_tensor("wv", [D, NPC], BF, kind="ExternalInput").ap()
    wo_d = nc.dram_tensor("wo", [NPC, D], BF, kind="ExternalInput").ap()
    bq_d = nc.dram_tensor("bq", [NT, 128, 1], FP32, kind="ExternalInput").ap()
    o_d = nc.dram_tensor("o", [S, D], FP32, kind="ExternalOutput").ap()

    with tile.TileContext(nc) as tc:
        with (
            tc.tile_pool(name="consts", bufs=1) as consts,
            tc.tile_pool(name="acts", bufs=1) as acts,
            tc.tile_pool(name="zp", bufs=2) as zp,
            tc.tile_pool(name="up", bufs=6) as up,
            tc.tile_pool(name="zbp", bufs=2) as zbp,
            tc.tile_pool(name="osb", bufs=4) as osb,
        ):
            # ---- resident weights ----
            wq_sb = consts.tile([128, KT, NPC], BF, tag="wq")
            nc.sync.dma_start(wq_sb[:], wq_d.rearrange("(t p) n -> p t n", p=128))
            wk_sb = consts.tile([128, KT, NPC], BF, tag="wk")
            nc.sync.dma_start(wk_sb[:], wk_d.rearrange("(t p) n -> p t n", p=128))
            wv_sb = consts.tile([128, KT, NPC], BF, tag="wv")
            nc.sync.dma_start(wv_sb[:], wv_d.rearrange("(t p) n -> p t n", p=128))
            wo_sb = consts.tile([128, NT, D], BF, tag="wo")
            nc.sync.dma_start(wo_sb[:], wo_d.rearrange("(t p) n -> p t n", p=128))
            bq_sb = consts.tile([128, NT, 1], FP32, tag="bq")
            nc.sync.dma_start(bq_sb[:], bq_d.rearrange("t p o -> p t o"))

            # ---- activations kept resident ----
            QT_sb = acts.tile([128, NT, S], BF, tag="QT")    # [head_dim, m]
            KT_sb = acts.tile([128, NT, S], BF, tag="KT")    # [head_dim, j]
            v_sb = acts.tile([128, JT, HPC, KEY + 1], BF, tag="v")  # V' + ones col
            ctxT_sb = acts.tile([128, NT, S], BF, tag="ctxT")

            nc.vector.memset(v_sb[:, :, :, KEY:KEY + 1], 1.0)

            # ================= single PSUM pool =================
            # "ss": 2x[128,1024] (4 banks) scores / out-proj
            # "cc": 4x[128,512] (4 banks) proj chains, V chains, ctx chains
            # ctx for phase i is PHASE-SHIFTED: its MMs run during phase i+1,
            # so cc is free during phase 0 to absorb V / K-nt1 / Q-nt1.
            with (
                tc.tile_pool(name="expp", bufs=34) as expp,
                tc.tile_pool(name="xep", bufs=1) as xep,
                tc.tile_pool(name="xdp", bufs=3) as xdp,
                tc.tile_pool(name="ps", bufs=2, space="PSUM") as ps,
                tc.tile_pool(name="cp", bufs=4, space="PSUM") as cp,
            ):
                xeT_sb = xep.tile([128, KT, S], BF, tag="xeT")
                for kt in range(KT):
                    nc.sync.dma_start(
                        xeT_sb[:, kt, :],
                        xeT.rearrange("(t p) m -> p t m", p=128)[:, kt, :],
                    )
                xdT_r = xdT.rearrange("(t p) m -> p t m", p=128)

                def q_chunk(kt, eng=None):
                    t = xdp.tile([128, S], BF, tag="xd", name=f"xdc_{kt}")
                    nc.sync.dma_start(t[:], xdT_r[:, kt, :])
                    return t

                def proj_pass(w_sb, nt, chains, x_tiles, kt):
                    for mc in range(MC):
                        nc.tensor.matmul(
                            chains[mc],
                            w_sb[:, kt, nt * 128:(nt + 1) * 128],
                            x_tiles[mc // 4] if isinstance(x_tiles, list)
                            else x_tiles[:, kt, mc * 512:(mc + 1) * 512],
                            start=(kt == 0),
                            stop=(kt == KT - 1),
                        )

                def evict_proj(chains, dst, nt, bias):
                    for mc in range(MC):
                        out_ap = dst[:, nt, mc * 512:(mc + 1) * 512]
                        if bias is not None:
                            nc.vector.tensor_scalar_add(out_ap, chains[mc], bias[:, nt, :])
                        else:
                            nc.vector.tensor_copy(out_ap, chains[mc])

                # ---- prefix: K-nt0 then Q-nt0 (heads 0,1) ----
                k0 = [cp.tile([128, 512], FP32, tag="cc", name=f"k0_{i}") for i in range(4)]
                for kt in range(KT):
                    for mc in range(MC):
                        nc.tensor.matmul(
                            k0[mc][:], wk_sb[:, kt, 0:128],
                            xeT_sb[:, kt, mc * 512:(mc + 1) * 512],
                            start=(kt == 0), stop=(kt == KT - 1),
                        )
                evict_proj([c[:] for c in k0], KT_sb, 0, None)
                q0 = [cp.tile([128, 512], FP32, tag="cc", name=f"q0_{i}") for i in range(4)]
                for kt in range(KT):
                    xc = q_chunk(kt)
                    for mc in range(MC):
                        nc.tensor.matmul(
                            q0[mc][:], wq_sb[:, kt, 0:128],
                            xc[:, mc * 512:(mc + 1) * 512],
                            start=(kt == 0), stop=(kt == KT - 1),
                        )
                evict_proj([c[:] for c in q0], QT_sb, 0, bq_sb)

                # ---- phases: scores(si) + shifted ctx(si-1) + dribbles ----
                order = [(0, 0), (0, 1), (1, 0), (1, 1)]
                rows = [0, KEY]
                prev = None  # (hp, mh, exp_tiles)
                drib = {}    # state for phase-0 dribbles

                def emit_ctx_step(hp, mh, jt, exp_row, ccs):
                    for hh in range(2):
                        h = hp * 2 + hh
                        for q in range(2):
                            nc.tensor.matmul(
                                ccs[hh * 2 + q][0:KEY + 1, :],
                                v_sb[:, jt, h, :],
                                exp_row[hh][:, q * 512:(q + 1) * 512],
                                start=(jt == 0),
                                stop=(jt == JT - 1),
                            )

                def emit_norm(hp, mh, ccs):
                    m0 = mh * 1024
                    for hh in range(2):
                        row = rows[hh]
                        for q in range(2):
                            c = ccs[hh * 2 + q]
                            u = up.tile([KEY + 1, 512], FP32, tag="u")
                            nc.vector.tensor_copy(u[:], c[0:KEY + 1, :])
                            zraw = zp.tile([1, 512], FP32, tag="z")
                            nc.vector.tensor_copy(zraw[:], u[KEY:KEY + 1, :])
                            zb = zbp.tile([KEY, 512], FP32, tag="zb")
                            nc.gpsimd.partition_broadcast(zb[:], zraw[:])
                            zbr = zbp.tile([KEY, 512], FP32, tag="zbr")
                            nc.vector.reciprocal_approx_fast(zbr[:], zb[:])
                            nc.vector.tensor_mul(
                                ctxT_sb[row:row + KEY, hp, m0 + q * 512:m0 + (q + 1) * 512],
                                u[0:KEY, :],
                                zbr[:],
                            )

                def emit_phase0_dribble(jt):
                    # jt 0-7: V pairs (ctx of phase 0 needs them in
                    # phase 1); jt 8-11: K-nt1; jt 12-15: Q-nt1
                    # (nt1 first needed by phase 2's scores).
                    if jt < 8:
                        jt0 = jt * 2
                        pv = [cp.tile([128, 512], FP32, tag="cc",
                                      name=f"pv_{jt0}_{d}") for d in range(2)]
                        for kt in range(KT):
                            for d in range(2):
                                nc.tensor.matmul(
                                    pv[d][:, 0:NPC],
                                    xeT_sb[:, kt, (jt0 + d) * 128:(jt0 + d + 1) * 128],
                                    wv_sb[:, kt, :],
                                    start=(kt == 0), stop=(kt == KT - 1),
                                )
                        for d in range(2):
                            nc.vector.tensor_copy(
                                v_sb[:, jt0 + d, :, 0:KEY],
                                pv[d][:, 0:NPC].rearrange("p (h k) -> p h k", h=HPC),
                            )
                    elif jt < 12:
                        if jt == 8:
                            drib["k1"] = [cp.tile([128, 512], FP32, tag="cc",
                                                  name=f"k1_{i}") for i in range(4)]
                        for kk in range(2):
                            kt = (jt - 8) * 2 + kk
                            for mc in range(MC):
                                nc.tensor.matmul(
                                    drib["k1"][mc][:], wk_sb[:, kt, 128:256],
                                    xeT_sb[:, kt, mc * 512:(mc + 1) * 512],
                                    start=(kt == 0), stop=(kt == KT - 1),
                                )
                        if jt == 11:
                            evict_proj([c[:] for c in drib["k1"]], KT_sb, 1, None)
                    else:
                        if jt == 12:
                            drib["q1"] = [cp.tile([128, 512], FP32, tag="cc",
                                                  name=f"q1_{i}") for i in range(4)]
                        for kk in range(2):
                            kt = (jt - 12) * 2 + kk
                            xc = q_chunk(kt)
                            for mc in range(MC):
                                nc.tensor.matmul(
                                    drib["q1"][mc][:], wq_sb[:, kt, 128:256],
                                    xc[:, mc * 512:(mc + 1) * 512],
                                    start=(kt == 0), stop=(kt == KT - 1),
                                )
                        if jt == 15:
                            evict_proj([c[:] for c in drib["q1"]], QT_sb, 1, bq_sb)

                for si, (hp, mh) in enumerate(order):
                    m0 = mh * 1024
                    last = si == len(order) - 1
                    cur_cc = None
                    prev_cc = None
                    if prev is not None:
                        prev_cc = [cp.tile([128, 512], FP32, tag="cc", name=f"cc_{si}_{i}")
                                   for i in range(4)]
                    cur_exps = []
                    for jt in range(JT):
                        # ctx/dribble MMs are emitted BEFORE the scores pair:
                        # the PE chews on them while the second ss buffer is
                        # still being drained by ACT, so that both K=64
                        # head-half matmuls become ready together and stream
                        # concurrently in PE row-groups (0,0)/(64,0).
                        if prev is not None and not last:
                            emit_ctx_step(prev[0], prev[1], jt, prev[2][jt], prev_cc)
                        if si == 0:
                            emit_phase0_dribble(jt)
                        exp_row = []
                        sst = [ps.tile([128, 1024], FP32, tag="ss",
                                       name=f"ss_{si}_{jt}_{hh2}") for hh2 in range(2)]
                        for q in range(2):
                            for hh in range(2):
                                row = rows[hh]
                                nc.tensor.matmul(
                                    sst[hh][:, q * 512:(q + 1) * 512],
                                    KT_sb[row:row + KEY, hp, jt * 128:(jt + 1) * 128],
                                    QT_sb[row:row + KEY, hp, m0 + q * 512:m0 + (q + 1) * 512],
                                    start=True, stop=True,
                                )
                        for hh in range(2):
                            et = expp.tile([128, 1024], BF, tag="exp")
                            nc.scalar.activation(et[:], sst[hh][:], AF.Exp, scale=0.125)
                            exp_row.append(et)
                        cur_exps.append(exp_row)
                        if last:
                            # front half: finish prev phase's ctx at 2x rate;
                            # back half: this phase's own ctx at 2x rate (its
                            # exp tiles exist by then) -> nothing left for the
                            # tail but normalization + output projection.
                            if jt < 8:
                                for j2 in (jt * 2, jt * 2 + 1):
                                    emit_ctx_step(prev[0], prev[1], j2, prev[2][j2], prev_cc)
                                if jt == 7:
                                    emit_norm(prev[0], prev[1], prev_cc)
                            else:
                                if jt == 8:
                                    cur_cc = [cp.tile([128, 512], FP32, tag="cc",
                                                      name=f"cc_last_{i}") for i in range(4)]
                                for j2 in ((jt - 8) * 2, (jt - 8) * 2 + 1):
                                    emit_ctx_step(hp, mh, j2, cur_exps[j2], cur_cc)
                    if last:
                        emit_norm(hp, mh, cur_cc)
                    elif prev is not None:
                        emit_norm(prev[0], prev[1], prev_cc)
                    prev = (hp, mh, cur_exps)

                # ================= output projection =================
                for mt in range(S // 128):
                    ot = osb.tile([128, D], FP32, tag="ot")
                    po = ps.tile([128, 1024], FP32, tag="ss", name=f"po_{mt}")
                    for dt in range(NT):
                        for ec in range(2):
                            nc.tensor.matmul(
                                po[:, ec * 512:(ec + 1) * 512],
                                ctxT_sb[:, dt, mt * 128:(mt + 1) * 128],
                                wo_sb[:, dt, ec * 512:(ec + 1) * 512],
                                start=(dt == 0),
                                stop=(dt == NT - 1),
                            )
                    nc.scalar.copy(ot[:], po[:])
                    nc.sync.dma_start(o_d[mt * 128:(mt + 1) * 128, :], ot[:])

    nc.compile()
    return nc


def _get_nc():
    global _NC
    if _NC is None:
        _NC = _build_nc()
    return _NC


def _maybe_register_ntff_hook():
    """Optional: register the axon NTFF profile hook so BASS_TRACE=1 yields
    HW exec times. No-op if unavailable (e.g. the grading environment)."""
    if "antenv.axon_hooks" in sys.modules:
        return
    try:
        import types

        if "/root/.axon_site" not in sys.path and os.path.isdir("/root/.axon_site"):
            sys.path.append("/root/.axon_site")
        from trn_agent_boot.trn_boot import _ntff_profile_via_ctypes

        hook = _ntff_profile_via_ctypes("/opt/axon/libaxon_pjrt.so")
        mod = types.ModuleType("antenv.axon_hooks")
        mod.get_axon_ntff_profile_hook = lambda: hook
        mod.set_axon_ntff_profile_hook = lambda h: None
        sys.modules["antenv.axon_hooks"] = mod
    except Exception:
        pass


def kernel(decoder_output, encoder_output, wq, bq, wk, bk, wv, bv, wo, bo):
    from concourse.bass_utils import run_bass_kernel_spmd

    global LAST_RESULTS

    decoder_output = np.asarray(decoder_output, dtype=np.float32)
    encoder_output = np.asarray(encoder_output, dtype=np.float32)
    wq = np.asarray(wq, dtype=np.float32)
    wk = np.asarray(wk, dtype=np.float32)
    wv = np.asarray(wv, dtype=np.float32)
    wo = np.asarray(wo, dtype=np.float32)
    bq = np.asarray(bq, dtype=np.float32)
    bv = np.asarray(bv, dtype=np.float32)
    bo = np.asarray(bo, dtype=np.float32)
    # bk is softmax-invariant (adds a per-query constant to every logit).

    if os.environ.get("BASS_TRACE"):
        _maybe_register_ntff_hook()

    nc = _get_nc()

    xT = {}
    for b in range(B):
        xT[("d", b)] = np.ascontiguousarray(decoder_output[b].T).astype(BF16)
        xT[("e", b)] = np.ascontiguousarray(encoder_output[b].T).astype(BF16)

    in_maps = []
    for c in range(8):
        b, hg = c // 4, c % 4
        sl = slice(hg * NPC, (hg + 1) * NPC)
        in_maps.append({
            "xdT": xT[("d", b)],
            "xeT": xT[("e", b)],
            "wq": wq[:, sl].astype(BF16),
            "wk": wk[:, sl].astype(BF16),
            "wv": wv[:, sl].astype(BF16),
            "wo": np.ascontiguousarray(wo[sl, :]).astype(BF16),
            "bq": bq[sl].reshape(NT, 128, 1),
        })

    res = run_bass_kernel_spmd(nc, in_maps, core_ids=list(range(8)))
    LAST_RESULTS = res

    correction = (bv @ wo + bo).astype(np.float32)  # probs sum to 1
    out = np.zeros((B, S, D), dtype=np.float32)
    for c in range(8):
        out[c // 4] += res.results[c]["o"]
    out += correction[None, None, :]
    return out



# revision 9
# speedup vs baseline: 1.0555x; 1.0483x over previous
"""Self-contained Trainium2 Bass kernel for 16-head cross-attention MHA.

Problem: B=2, SQ=SK=2048, D=1024, H=16, key_size=64 (fp32 in/out).

Sharding (8 cores): data-parallel over batch (2) x tensor-parallel over
head groups (4 heads per core). Each core computes its 4 heads'
Q/K/V projections (column slices of wq/wk/wv), attention, and a partial
output projection (row slice of wo), uploaded bf16. Host sums the 4
partial outputs per batch and adds the (bv @ wo + bo) correction (probs
sum to 1, so bv contributes exactly bv @ wo; bk cancels in softmax).

Device pipeline per core (bf16 matmuls, fp32 PSUM accumulation), ACT
(ScalarE exp over 4 x 2048 x 2048 scores, ~147us) is the pacer:

  1. Inputs stream in 512-column blocks (wk/wq first) so the first
     score matmul issues ~10us in; the K/Q/V projections for later
     blocks dribble through phase slack.
  2. Score PSUM tiles are laid out per q-chunk as [hh0 | hh1] so one
     exp ACT call drains (and releases) both head-halves together:
     the two K=64 score matmuls of the next tile become ready
     simultaneously and stream CONCURRENTLY in PE row-groups
     (0,0)/(64,0) (tile_position auto-derived from base partitions),
     halving score PE time.
  3. Phases run (hp, mh) = (0,0),(1,0),(0,1),(1,1). ctx for phase i is
     phase-shifted into phase i+1 (PSUM: "ss" 2x[128,1024] q-chunk
     score tiles + "cc" 4x[128,512] chains; V' carries a ones column so
     row 64 accumulates the softmax denominator Z).
  4. ctx(p1) runs 2x-rate in p2's front half, freeing p2's back half
     for the m-half-0 output projection (ctx for m-half 0 is complete
     and normalized by then). ctx(p2) runs 2x in p3's front half,
     ctx(p3) 2x in its back half, leaving only norm + the m-half-1
     output projection for the tail.
  5. Normalization: Z row partition-broadcast via GpSimd, DVE
     reciprocal, DVE multiply into ctx^T bf16.
"""

import os
import sys

for _p in ("/opt/trn_rl_repo", "/root/.axon_site/_ro/trn_rl_repo"):
    if os.path.isdir(_p) and _p not in sys.path:
        sys.path.insert(0, _p)

import numpy as np
import ml_dtypes

BF16 = ml_dtypes.bfloat16

B = 2
S = 2048          # SQ == SK
D = 1024
H = 16
KEY = 64
HPC = 4           # heads per core
NPC = HPC * KEY   # 256 per-core slice of D
KT = D // 128     # 8 contraction tiles for projections
NT = NPC // 128   # 2 head-pair tiles
MC = S // 512     # 4 m-chunks of 512
JT = S // 128     # 16 key tiles
NB = 4            # 512-wide column blocks for streaming/projection

_NC = None
LAST_RESULTS = None  # BassKernelResults of the most recent run (for test.py)


def _build_nc():
    import concourse.tile as tile
    from concourse import bacc, mybir

    FP32 = mybir.dt.float32
    BF = mybir.dt.bfloat16
    AF = mybir.ActivationFunctionType

    nc = bacc.Bacc("TRN2", target_bir_lowering=False, debug=False, num_devices=8)

    xdT = nc.dram_tensor("xdT", [D, S], BF, kind="ExternalInput").ap()
    xeT = nc.dram_tensor("xeT", [D, S], BF, kind="ExternalInput").ap()
    wq_d = nc.dram_tensor("wq", [D, NPC], BF, kind="ExternalInput").ap()
    wk_d = nc.dram_tensor("wk", [D, NPC], BF, kind="ExternalInput").ap()
    wv_d = nc.dram_tensor("wv", [D, NPC], BF, kind="ExternalInput").ap()
    wo_d = nc.dram_tensor("wo", [NPC, D], BF, kind="ExternalInput").ap()
    bq_d = nc.dram_tensor("bq", [NT, 128, 1], FP32, kind="ExternalInput").ap()
    o_d = nc.dram_tensor("o", [S, D], BF, kind="ExternalOutput").ap()

    xeT_r = xeT.rearrange("(t p) m -> p t m", p=128)
    xdT_r = xdT.rearrange("(t p) m -> p t m", p=128)

    with tile.TileContext(nc) as tc:
        with (
            tc.tile_pool(name="consts", bufs=1) as consts,
            tc.tile_pool(name="acts", bufs=1) as acts,
            tc.tile_pool(name="zp", bufs=2) as zp,
            tc.tile_pool(name="up", bufs=6) as up,
            tc.tile_pool(name="zbp", bufs=2) as zbp,
            tc.tile_pool(name="osb", bufs=4) as osb,
        ):
            # ---- resident weights (wk/wq first: prefix critical path) ----
            wk_sb = consts.tile([128, KT, NPC], BF, tag="wk")
            nc.sync.dma_start(wk_sb[:], wk_d.rearrange("(t p) n -> p t n", p=128))
            wq_sb = consts.tile([128, KT, NPC], BF, tag="wq")
            nc.sync.dma_start(wq_sb[:], wq_d.rearrange("(t p) n -> p t n", p=128))
            wv_sb = consts.tile([128, KT, NPC], BF, tag="wv")
            wo_sb = consts.tile([128, NT, D], BF, tag="wo")
            bq_sb = consts.tile([128, NT, 1], FP32, tag="bq")

            # ---- activations kept resident ----
            QT_sb = acts.tile([128, NT, S], BF, tag="QT")    # [head_dim, m]
            KT_sb = acts.tile([128, NT, S], BF, tag="KT")    # [head_dim, j]
            v_sb = acts.tile([128, JT, HPC, KEY + 1], BF, tag="v")  # V' + ones
            ctxT_sb = acts.tile([128, NT, S], BF, tag="ctxT")

            nc.vector.memset(v_sb[:, :, :, KEY:KEY + 1], 1.0)

            # PSUM: "ss" 2x[128,1024] (4 banks) q-chunk score tiles
            #       "cc" 4x[128,512] (4 banks) ctx/proj/out-proj chains
            with (
                tc.tile_pool(name="expp", bufs=34) as expp,
                tc.tile_pool(name="xep", bufs=1) as xep,
                tc.tile_pool(name="xdp", bufs=2) as xdp,
                tc.tile_pool(name="ps", bufs=2, space="PSUM") as ps,
                tc.tile_pool(name="cp", bufs=4, space="PSUM") as cp,
            ):
                # ---- streamed inputs: 512-col blocks; block 0 per-kt ----
                xeT_sb = xep.tile([128, KT, S], BF, tag="xeT")
                for kt in range(KT):
                    nc.sync.dma_start(xeT_sb[:, kt, 0:512], xeT_r[:, kt, 0:512])
                xd_tiles = []
                xb0 = xdp.tile([128, KT, 512], BF, tag="xd", name="xdb_0")
                xd_tiles.append(xb0)
                for kt in range(KT):
                    nc.sync.dma_start(xb0[:, kt, :], xdT_r[:, kt, 0:512])
                for b in range(1, NB):
                    nc.sync.dma_start(
                        xeT_sb[:, :, b * 512:(b + 1) * 512],
                        xeT_r[:, :, b * 512:(b + 1) * 512],
                    )
                xb1 = xdp.tile([128, KT, 512], BF, tag="xd", name="xdb_1")
                xd_tiles.append(xb1)
                nc.sync.dma_start(xb1[:], xdT_r[:, :, 512:1024])
                # later-needed weights after the prefix-critical loads
                nc.sync.dma_start(wv_sb[:], wv_d.rearrange("(t p) n -> p t n", p=128))
                nc.sync.dma_start(bq_sb[:], bq_d.rearrange("t p o -> p t o"))
                nc.sync.dma_start(wo_sb[:], wo_d.rearrange("(t p) n -> p t n", p=128))
                # xd blocks 2,3 reuse the 2 xd buffers (WAR-gated until the
                # nt1 Q chains of blocks 0,1 finish in phase-0 dribbles)
                for b in range(2, NB):
                    xb = xdp.tile([128, KT, 512], BF, tag="xd", name=f"xdb_{b}")
                    xd_tiles.append(xb)
                    nc.sync.dma_start(xb[:], xdT_r[:, :, b * 512:(b + 1) * 512])

                # ---- projection work units (chain + evict) ----
                def k_unit(b, nt):
                    ch = cp.tile([128, 512], FP32, tag="cc", name=f"kch_{b}_{nt}")
                    for kt in range(KT):
                        nc.tensor.matmul(
                            ch[:], wk_sb[:, kt, nt * 128:(nt + 1) * 128],
                            xeT_sb[:, kt, b * 512:(b + 1) * 512],
                            start=(kt == 0), stop=(kt == KT - 1),
                        )
                    nc.vector.tensor_copy(KT_sb[:, nt, b * 512:(b + 1) * 512], ch[:])

                def q_unit(b, nt):
                    ch = cp.tile([128, 512], FP32, tag="cc", name=f"qch_{b}_{nt}")
                    for kt in range(KT):
                        nc.tensor.matmul(
                            ch[:], wq_sb[:, kt, nt * 128:(nt + 1) * 128],
                            xd_tiles[b][:, kt, :],
                            start=(kt == 0), stop=(kt == KT - 1),
                        )
                    nc.vector.tensor_scalar_add(
                        QT_sb[:, nt, b * 512:(b + 1) * 512], ch[:], bq_sb[:, nt, :])

                def v_unit(u):
                    # key tiles 2u, 2u+1: V' rows for all 4 heads
                    for dd in range(2):
                        jv = 2 * u + dd
                        ch = cp.tile([128, 512], FP32, tag="cc", name=f"vch_{jv}")
                        for kt in range(KT):
                            nc.tensor.matmul(
                                ch[:, 0:NPC],
                                xeT_sb[:, kt, jv * 128:(jv + 1) * 128],
                                wv_sb[:, kt, :],
                                start=(kt == 0), stop=(kt == KT - 1),
                            )
                        nc.vector.tensor_copy(
                            v_sb[:, jv, :, 0:KEY],
                            ch[:, 0:NPC].rearrange("p (h k) -> p h k", h=HPC),
                        )

                def outproj_unit(mt):
                    ot = osb.tile([128, D], BF, tag="ot", name=f"ot_{mt}")
                    for ec in range(2):
                        po = cp.tile([128, 512], FP32, tag="cc",
                                     name=f"po_{mt}_{ec}")
                        for dt in range(NT):
                            nc.tensor.matmul(
                                po[:],
                                ctxT_sb[:, dt, mt * 128:(mt + 1) * 128],
                                wo_sb[:, dt, ec * 512:(ec + 1) * 512],
                                start=(dt == 0), stop=(dt == NT - 1),
                            )
                        nc.vector.tensor_copy(ot[:, ec * 512:(ec + 1) * 512], po[:])
                    nc.sync.dma_start(o_d[mt * 128:(mt + 1) * 128, :], ot[:])

                # ---- prefix: block-0/1 K and Q (nt0) so scores start ASAP
                k_unit(0, 0)
                q_unit(0, 0)
                q_unit(1, 0)
                k_unit(1, 0)

                # dribble schedules: phase -> {jt: emit_fn}
                p0_drib = {
                    0: lambda: k_unit(2, 0), 1: lambda: k_unit(3, 0),
                    2: lambda: k_unit(0, 1), 3: lambda: v_unit(0),
                    4: lambda: k_unit(1, 1), 5: lambda: v_unit(1),
                    6: lambda: k_unit(2, 1), 7: lambda: v_unit(2),
                    8: lambda: k_unit(3, 1), 9: lambda: v_unit(3),
                    10: lambda: q_unit(0, 1), 11: lambda: v_unit(4),
                    12: lambda: q_unit(1, 1), 13: lambda: v_unit(5),
                    14: lambda: v_unit(6), 15: lambda: v_unit(7),
                }
                p1_drib = {
                    0: lambda: q_unit(2, 0), 2: lambda: q_unit(3, 0),
                    4: lambda: q_unit(2, 1), 6: lambda: q_unit(3, 1),
                }

                rows = [0, KEY]

                def emit_scores(si, hp, m0, jt, cur_exps):
                    # two q-chunk tiles, each [hh0 | hh1]; the two K=64
                    # matmuls per chunk stream concurrently (row groups)
                    sst = [ps.tile([128, 1024], FP32, tag="ss",
                                   name=f"ss_{si}_{jt}_{q2}") for q2 in range(2)]
                    for q in range(2):
                        for hh in range(2):
                            row = rows[hh]
                            nc.tensor.matmul(
                                sst[q][:, hh * 512:(hh + 1) * 512],
                                KT_sb[row:row + KEY, hp, jt * 128:(jt + 1) * 128],
                                QT_sb[row:row + KEY, hp,
                                      m0 + q * 512:m0 + (q + 1) * 512],
                                start=True, stop=True,
                            )
                    pair = []
                    for q in range(2):
                        et = expp.tile([128, 1024], BF, tag="exp")
                        nc.scalar.activation(et[:], sst[q][:], AF.Exp, scale=0.125)
                        pair.append(et)
                    cur_exps.append(pair)

                def emit_ctx_step(hp, jt, exp_pair, ccs):
                    for hh in range(2):
                        h = hp * 2 + hh
                        for q in range(2):
                            nc.tensor.matmul(
                                ccs[hh * 2 + q][0:KEY + 1, :],
                                v_sb[:, jt, h, :],
                                exp_pair[q][:, hh * 512:(hh + 1) * 512],
                                start=(jt == 0),
                                stop=(jt == JT - 1),
                            )

                def emit_norm(hp, mh, ccs):
                    m0n = mh * 1024
                    for hh in range(2):
                        row = rows[hh]
                        for q in range(2):
                            c = ccs[hh * 2 + q]
                            u = up.tile([KEY + 1, 512], FP32, tag="u")
                            nc.vector.tensor_copy(u[:], c[0:KEY + 1, :])
                            zraw = zp.tile([1, 512], FP32, tag="z")
                            nc.vector.tensor_copy(zraw[:], u[KEY:KEY + 1, :])
                            zb = zbp.tile([KEY, 512], FP32, tag="zb")
                            nc.gpsimd.partition_broadcast(zb[:], zraw[:])
                            zbr = zbp.tile([KEY, 512], FP32, tag="zbr")
                            nc.vector.reciprocal_approx_fast(zbr[:], zb[:])
                            nc.vector.tensor_mul(
                                ctxT_sb[row:row + KEY, hp,
                                        m0n + q * 512:m0n + (q + 1) * 512],
                                u[0:KEY, :],
                                zbr[:],
                            )

                # ---- phases ----
                order = [(0, 0), (1, 0), (0, 1), (1, 1)]
                prev = None  # (hp, mh, exps)

                for si, (hp, mh) in enumerate(order):
                    m0 = mh * 1024
                    cur_exps = []
                    cc = None
                    cur_cc = None
                    if prev is not None:
                        cc = [cp.tile([128, 512], FP32, tag="cc",
                                      name=f"cc_{si}_{i}") for i in range(4)]
                    for jt in range(JT):
                        emit_scores(si, hp, m0, jt, cur_exps)
                        if si == 1:
                            # ctx(p0) at 1x
                            emit_ctx_step(prev[0], jt, prev[2][jt], cc)
                        elif si >= 2:
                            # ctx(prev) at 2x in the front half
                            if jt < 8:
                                for j2 in (jt * 2, jt * 2 + 1):
                                    emit_ctx_step(prev[0], j2, prev[2][j2], cc)
                                if jt == 7:
                                    emit_norm(prev[0], prev[1], cc)
                            elif si == 2:
                                # back half: m-half-0 output projection
                                # (ctx(0,0) and ctx(1,0) both normalized)
                                outproj_unit(jt - 8)
                            else:
                                # si == 3: own ctx at 2x in the back half
                                if jt == 8:
                                    cur_cc = [cp.tile([128, 512], FP32, tag="cc",
                                                      name=f"cc_last_{i}")
                                              for i in range(4)]
                                for j2 in ((jt - 8) * 2, (jt - 8) * 2 + 1):
                                    emit_ctx_step(hp, j2, cur_exps[j2], cur_cc)
                        if si == 0 and jt in p0_drib:
                            p0_drib[jt]()
                        elif si == 1 and jt in p1_drib:
                            p1_drib[jt]()
                    if si == 1:
                        emit_norm(prev[0], prev[1], cc)
                    prev = (hp, mh, cur_exps)

                # ---- tail: last norm + m-half-1 output projection ----
                emit_norm(1, 1, cur_cc)
                for mt in range(8, 16):
                    outproj_unit(mt)

    nc.compile()
    return nc


def _get_nc():
    global _NC
    if _NC is None:
        _NC = _build_nc()
    return _NC


def _maybe_register_ntff_hook():
    """Optional: register the axon NTFF profile hook so BASS_TRACE=1 yields
    HW exec times. No-op if unavailable (e.g. the grading environment)."""
    if "antenv.axon_hooks" in sys.modules:
        return
    try:
        import types

        if "/root/.axon_site" not in sys.path and os.path.isdir("/root/.axon_site"):
            sys.path.append("/root/.axon_site")
        from trn_agent_boot.trn_boot import _ntff_profile_via_ctypes

        hook = _ntff_profile_via_ctypes("/opt/axon/libaxon_pjrt.so")
        mod = types.ModuleType("antenv.axon_hooks")
        mod.get_axon_ntff_profile_hook = lambda: hook
        mod.set_axon_ntff_profile_hook = lambda h: None
        sys.modules["antenv.axon_hooks"] = mod
    except Exception:
        pass


def kernel(decoder_output, encoder_output, wq, bq, wk, bk, wv, bv, wo, bo):
    from concourse.bass_utils import run_bass_kernel_spmd

    global LAST_RESULTS

    decoder_output = np.asarray(decoder_output, dtype=np.float32)
    encoder_output = np.asarray(encoder_output, dtype=np.float32)
    wq = np.asarray(wq, dtype=np.float32)
    wk = np.asarray(wk, dtype=np.float32)
    wv = np.asarray(wv, dtype=np.float32)
    wo = np.asarray(wo, dtype=np.float32)
    bq = np.asarray(bq, dtype=np.float32)
    bv = np.asarray(bv, dtype=np.float32)
    bo = np.asarray(bo, dtype=np.float32)
    # bk is softmax-invariant (adds a per-query constant to every logit).

    if os.environ.get("BASS_TRACE"):
        _maybe_register_ntff_hook()

    nc = _get_nc()

    xT = {}
    for b in range(B):
        xT[("d", b)] = np.ascontiguousarray(decoder_output[b].T).astype(BF16)
        xT[("e", b)] = np.ascontiguousarray(encoder_output[b].T).astype(BF16)

    in_maps = []
    for c in range(8):
        b, hg = c // 4, c % 4
        sl = slice(hg * NPC, (hg + 1) * NPC)
        in_maps.append({
            "xdT": xT[("d", b)],
            "xeT": xT[("e", b)],
            "wq": wq[:, sl].astype(BF16),
            "wk": wk[:, sl].astype(BF16),
            "wv": wv[:, sl].astype(BF16),
            "wo": np.ascontiguousarray(wo[sl, :]).astype(BF16),
            "bq": bq[sl].reshape(NT, 128, 1),
        })

    res = run_bass_kernel_spmd(nc, in_maps, core_ids=list(range(8)))
    LAST_RESULTS = res

    correction = (bv @ wo + bo).astype(np.float32)  # probs sum to 1
    out = np.zeros((B, S, D), dtype=np.float32)
    for c in range(8):
        out[c // 4] += np.asarray(res.results[c]["o"]).astype(np.float32)
    out += correction[None, None, :]
    return out


# revision 18
# speedup vs baseline: 1.0625x; 1.0066x over previous
"""Self-contained Trainium2 Bass kernel for 16-head cross-attention MHA.

Problem: B=2, SQ=SK=2048, D=1024, H=16, key_size=64 (fp32 in/out).

Sharding (8 cores): data-parallel over batch (2) x tensor-parallel over
head groups (4 heads per core). Each core computes its 4 heads'
Q/K/V projections (column slices of wq/wk/wv), attention, and a partial
output projection (row slice of wo), uploaded bf16. Host sums the 4
partial outputs per batch and adds the (bv @ wo + bo) correction (probs
sum to 1, so bv contributes exactly bv @ wo; bk cancels in softmax).

Device pipeline per core (bf16 matmuls, fp32 PSUM accumulation), ACT
(ScalarE exp over 4 x 2048 x 2048 scores, ~147us) is the pacer:

  1. Inputs stream in 512-column blocks (wk/wq first) so the first
     score matmul issues ~10us in; the K/Q/V projections for later
     blocks dribble through phase slack.
  2. Score PSUM tiles are laid out per q-chunk as [hh0 | hh1] so one
     exp ACT call drains (and releases) both head-halves together:
     the two K=64 score matmuls of the next tile become ready
     simultaneously and stream CONCURRENTLY in PE row-groups
     (0,0)/(64,0) (tile_position auto-derived from base partitions),
     halving score PE time.
  3. Phases run (hp, mh) = (0,0),(1,0),(0,1),(1,1). ctx for phase i is
     phase-shifted into phase i+1 (PSUM: "ss" 2x[128,1024] q-chunk
     score tiles + "cc" 4x[128,512] chains; V' carries a ones column so
     row 64 accumulates the softmax denominator Z).
  4. ctx(p1) runs 2x-rate in p2's front half, freeing p2's back half
     for the m-half-0 output projection (ctx for m-half 0 is complete
     and normalized by then). ctx(p2) runs 2x in p3's front half,
     ctx(p3) 2x in its back half, leaving only norm + the m-half-1
     output projection for the tail.
  5. Normalization: Z row partition-broadcast via GpSimd, DVE
     reciprocal, DVE multiply into ctx^T bf16.
"""

import os
import sys

for _p in ("/opt/trn_rl_repo", "/root/.axon_site/_ro/trn_rl_repo"):
    if os.path.isdir(_p) and _p not in sys.path:
        sys.path.insert(0, _p)

import numpy as np
import ml_dtypes

BF16 = ml_dtypes.bfloat16

B = 2
S = 2048          # SQ == SK
D = 1024
H = 16
KEY = 64
HPC = 4           # heads per core
NPC = HPC * KEY   # 256 per-core slice of D
KT = D // 128     # 8 contraction tiles for projections
NT = NPC // 128   # 2 head-pair tiles
MC = S // 512     # 4 m-chunks of 512
JT = S // 128     # 16 key tiles
NB = 4            # 512-wide column blocks for streaming/projection

_NC = None
LAST_RESULTS = None  # BassKernelResults of the most recent run (for test.py)


def _build_nc():
    import concourse.tile as tile
    from concourse import bacc, mybir

    FP32 = mybir.dt.float32
    BF = mybir.dt.bfloat16
    AF = mybir.ActivationFunctionType

    nc = bacc.Bacc("TRN2", target_bir_lowering=False, debug=False, num_devices=8)

    # weights arrive host-preswizzled to [128, kt, n] so their DMAs read
    # 4KB-contiguous per partition
    xdT = nc.dram_tensor("xdT", [D, S], BF, kind="ExternalInput").ap()
    xeT = nc.dram_tensor("xeT", [D, S], BF, kind="ExternalInput").ap()
    wq_d = nc.dram_tensor("wq", [128, KT * NPC], BF, kind="ExternalInput").ap()
    wk_d = nc.dram_tensor("wk", [128, KT * NPC], BF, kind="ExternalInput").ap()
    wv_d = nc.dram_tensor("wv", [128, KT * NPC], BF, kind="ExternalInput").ap()
    wo_d = nc.dram_tensor("wo", [128, NT * D], BF, kind="ExternalInput").ap()
    bq_d = nc.dram_tensor("bq", [128, NT], FP32, kind="ExternalInput").ap()
    o_d = nc.dram_tensor("o", [S, D], BF, kind="ExternalOutput").ap()

    xeT_r = xeT.rearrange("(t p) m -> p t m", p=128)
    xdT_r = xdT.rearrange("(t p) m -> p t m", p=128)
    o_r = o_d.rearrange("(g p) n -> p g n", p=128)

    with tile.TileContext(nc) as tc:
        with (
            tc.tile_pool(name="consts", bufs=1) as consts,
            tc.tile_pool(name="acts", bufs=1) as acts,
            tc.tile_pool(name="zp", bufs=2) as zp,
            tc.tile_pool(name="up", bufs=6) as up,
            tc.tile_pool(name="zbp", bufs=2) as zbp,
            tc.tile_pool(name="osb", bufs=2) as osb,
        ):
            # ---- resident weights (wk/wq first: prefix critical path) ----
            wk_sb = consts.tile([128, KT, NPC], BF, tag="wk")
            nc.sync.dma_start(wk_sb[:], wk_d.rearrange("p (t n) -> p t n", t=KT))
            wq_sb = consts.tile([128, KT, NPC], BF, tag="wq")
            nc.sync.dma_start(wq_sb[:], wq_d.rearrange("p (t n) -> p t n", t=KT))
            wv_sb = consts.tile([128, KT, NPC], BF, tag="wv")
            wo_sb = consts.tile([128, NT, D], BF, tag="wo")
            bq_sb = consts.tile([128, NT, 1], FP32, tag="bq")

            # ---- activations kept resident ----
            QT_sb = acts.tile([128, NT, S], BF, tag="QT")    # [head_dim, m]
            KT_sb = acts.tile([128, NT, S], BF, tag="KT")    # [head_dim, j]
            v_sb = acts.tile([128, JT, HPC, KEY + 1], BF, tag="v")  # V' + ones
            ctxT_sb = acts.tile([128, NT, S], BF, tag="ctxT")

            nc.vector.memset(v_sb[:, :, :, KEY:KEY + 1], 1.0)

            # PSUM: "ss" 2x[128,1024] (4 banks) q-chunk score tiles
            #       "cc" 4x[128,512] (4 banks) ctx/proj/out-proj chains
            with (
                tc.tile_pool(name="expp", bufs=34) as expp,
                tc.tile_pool(name="xep", bufs=1) as xep,
                tc.tile_pool(name="xdp", bufs=2) as xdp,
                tc.tile_pool(name="ps", bufs=2, space="PSUM") as ps,
                tc.tile_pool(name="cp", bufs=4, space="PSUM") as cp,
            ):
                # ---- streamed inputs: 512-col blocks; block 0 per-kt ----
                xeT_sb = xep.tile([128, KT, S], BF, tag="xeT")
                for kt in range(KT):
                    nc.sync.dma_start(xeT_sb[:, kt, 0:512], xeT_r[:, kt, 0:512])
                xd_tiles = []
                xb0 = xdp.tile([128, KT, 512], BF, tag="xd", name="xdb_0")
                xd_tiles.append(xb0)
                for kt in range(KT):
                    nc.sync.dma_start(xb0[:, kt, :], xdT_r[:, kt, 0:512])
                nc.sync.dma_start(bq_sb[:], bq_d.rearrange("p (t o) -> p t o", o=1))
                xb1 = xdp.tile([128, KT, 512], BF, tag="xd", name="xdb_1")
                xd_tiles.append(xb1)
                nc.sync.dma_start(xb1[:], xdT_r[:, :, 512:1024])
                nc.sync.dma_start(
                    xeT_sb[:, :, 512:S], xeT_r[:, :, 512:S])
                # later-needed weights after the prefix-critical loads
                nc.sync.dma_start(wv_sb[:], wv_d.rearrange("p (t n) -> p t n", t=KT))
                nc.sync.dma_start(wo_sb[:], wo_d.rearrange("p (t n) -> p t n", t=NT))
                # xd blocks 2,3 reuse the 2 xd buffers (WAR-gated until the
                # nt1 Q chains of blocks 0,1 finish in phase-0 dribbles)
                for b in range(2, NB):
                    xb = xdp.tile([128, KT, 512], BF, tag="xd", name=f"xdb_{b}")
                    xd_tiles.append(xb)
                    nc.sync.dma_start(xb[:], xdT_r[:, :, b * 512:(b + 1) * 512])

                # ---- projection work units (chain + evict) ----
                def k_unit(b, nt):
                    ch = cp.tile([128, 512], FP32, tag="cc", name=f"kch_{b}_{nt}")
                    for kt in range(KT):
                        nc.tensor.matmul(
                            ch[:], wk_sb[:, kt, nt * 128:(nt + 1) * 128],
                            xeT_sb[:, kt, b * 512:(b + 1) * 512],
                            start=(kt == 0), stop=(kt == KT - 1),
                        )
                    nc.vector.tensor_copy(KT_sb[:, nt, b * 512:(b + 1) * 512], ch[:])

                def q_unit(b, nt):
                    ch = cp.tile([128, 512], FP32, tag="cc", name=f"qch_{b}_{nt}")
                    for kt in range(KT):
                        nc.tensor.matmul(
                            ch[:], wq_sb[:, kt, nt * 128:(nt + 1) * 128],
                            xd_tiles[b][:, kt, :],
                            start=(kt == 0), stop=(kt == KT - 1),
                        )
                    nc.vector.tensor_scalar_add(
                        QT_sb[:, nt, b * 512:(b + 1) * 512], ch[:], bq_sb[:, nt, :])

                def v_unit(u):
                    # key tiles 2u, 2u+1: V' rows for all 4 heads
                    for dd in range(2):
                        jv = 2 * u + dd
                        ch = cp.tile([128, 512], FP32, tag="cc", name=f"vch_{jv}")
                        for kt in range(KT):
                            nc.tensor.matmul(
                                ch[:, 0:NPC],
                                xeT_sb[:, kt, jv * 128:(jv + 1) * 128],
                                wv_sb[:, kt, :],
                                start=(kt == 0), stop=(kt == KT - 1),
                            )
                        nc.vector.tensor_copy(
                            v_sb[:, jv, :, 0:KEY],
                            ch[:, 0:NPC].rearrange("p (h k) -> p h k", h=HPC),
                        )

                oqueue = {}

                def outproj_unit(mt):
                    # staged in groups of 4 query-tiles -> one 1MB output DMA
                    g = mt // 4
                    if g not in oqueue:
                        oqueue[g] = osb.tile([128, 4, D], BF, tag="ot",
                                             name=f"ot_{g}")
                    ot = oqueue[g]
                    for ec in range(2):
                        po = cp.tile([128, 512], FP32, tag="cc",
                                     name=f"po_{mt}_{ec}")
                        for dt in range(NT):
                            nc.tensor.matmul(
                                po[:],
                                ctxT_sb[:, dt, mt * 128:(mt + 1) * 128],
                                wo_sb[:, dt, ec * 512:(ec + 1) * 512],
                                start=(dt == 0), stop=(dt == NT - 1),
                            )
                        nc.vector.tensor_copy(
                            ot[:, mt % 4, ec * 512:(ec + 1) * 512], po[:])
                    if mt % 4 == 3:
                        nc.sync.dma_start(o_r[:, g * 4:(g + 1) * 4, :], ot[:])

                # ---- prefix: block-0/1 K and Q (nt0) so scores start ASAP
                k_unit(0, 0)
                q_unit(0, 0)
                q_unit(1, 0)
                k_unit(1, 0)

                # dribble schedules: phase -> {jt: emit_fn}
                p0_drib = {
                    0: lambda: k_unit(2, 0), 1: lambda: k_unit(3, 0),
                    2: lambda: k_unit(0, 1), 3: lambda: v_unit(0),
                    4: lambda: k_unit(1, 1), 5: lambda: v_unit(1),
                    6: lambda: k_unit(2, 1), 7: lambda: v_unit(2),
                    8: lambda: k_unit(3, 1), 9: lambda: v_unit(3),
                    10: lambda: q_unit(0, 1), 11: lambda: v_unit(4),
                    12: lambda: q_unit(1, 1), 13: lambda: v_unit(5),
                    14: lambda: v_unit(6), 15: lambda: v_unit(7),
                }
                p1_drib = {
                    0: lambda: q_unit(2, 0), 2: lambda: q_unit(3, 0),
                    4: lambda: q_unit(2, 1), 6: lambda: q_unit(3, 1),
                }

                rows = [0, KEY]

                def emit_scores(si, hp, m0, jt, cur_exps):
                    # two q-chunk tiles, each [hh0 | hh1]; the two K=64
                    # matmuls per chunk stream concurrently (row groups)
                    sst = [ps.tile([128, 1024], FP32, tag="ss",
                                   name=f"ss_{si}_{jt}_{q2}") for q2 in range(2)]
                    for q in range(2):
                        for hh in range(2):
                            row = rows[hh]
                            nc.tensor.matmul(
                                sst[q][:, hh * 512:(hh + 1) * 512],
                                KT_sb[row:row + KEY, hp, jt * 128:(jt + 1) * 128],
                                QT_sb[row:row + KEY, hp,
                                      m0 + q * 512:m0 + (q + 1) * 512],
                                start=True, stop=True,
                            )
                    pair = []
                    for q in range(2):
                        et = expp.tile([128, 1024], BF, tag="exp")
                        nc.scalar.activation(et[:], sst[q][:], AF.Exp, scale=0.125)
                        pair.append(et)
                    cur_exps.append(pair)

                def emit_ctx_step(hp, jt, exp_pair, ccs):
                    for hh in range(2):
                        h = hp * 2 + hh
                        for q in range(2):
                            nc.tensor.matmul(
                                ccs[hh * 2 + q][0:KEY + 1, :],
                                v_sb[:, jt, h, :],
                                exp_pair[q][:, hh * 512:(hh + 1) * 512],
                                start=(jt == 0),
                                stop=(jt == JT - 1),
                            )

                def emit_norm(hp, mh, ccs):
                    m0n = mh * 1024
                    for hh in range(2):
                        row = rows[hh]
                        for q in range(2):
                            c = ccs[hh * 2 + q]
                            u = up.tile([KEY + 1, 512], FP32, tag="u")
                            nc.vector.tensor_copy(u[:], c[0:KEY + 1, :])
                            zraw = zp.tile([1, 512], FP32, tag="z")
                            nc.vector.tensor_copy(zraw[:], u[KEY:KEY + 1, :])
                            zb = zbp.tile([KEY, 512], FP32, tag="zb")
                            nc.gpsimd.partition_broadcast(zb[:], zraw[:])
                            zbr = zbp.tile([KEY, 512], FP32, tag="zbr")
                            nc.vector.reciprocal_approx_fast(zbr[:], zb[:])
                            nc.vector.tensor_mul(
                                ctxT_sb[row:row + KEY, hp,
                                        m0n + q * 512:m0n + (q + 1) * 512],
                                u[0:KEY, :],
                                zbr[:],
                            )

                # ---- phases ----
                order = [(0, 0), (1, 0), (0, 1), (1, 1)]
                prev = None  # (hp, mh, exps)

                for si, (hp, mh) in enumerate(order):
                    m0 = mh * 1024
                    cur_exps = []
                    cc = None
                    cur_cc = None
                    if prev is not None:
                        cc = [cp.tile([128, 512], FP32, tag="cc",
                                      name=f"cc_{si}_{i}") for i in range(4)]
                    for jt in range(JT):
                        emit_scores(si, hp, m0, jt, cur_exps)
                        if si == 1:
                            # ctx(p0), finishing 2 slots early so its norm
                            # overlaps the phase end and the chain buffers
                            # are free when p2's ctx(p1) starts
                            if jt < 14:
                                emit_ctx_step(prev[0], jt, prev[2][jt], cc)
                                if jt == 6:
                                    emit_ctx_step(prev[0], 14, prev[2][14], cc)
                                elif jt == 13:
                                    emit_ctx_step(prev[0], 15, prev[2][15], cc)
                            elif jt == 14:
                                emit_norm(prev[0], prev[1], cc)
                        elif si >= 2:
                            # ctx(prev) at 2x in the front half
                            if jt < 8:
                                for j2 in (jt * 2, jt * 2 + 1):
                                    emit_ctx_step(prev[0], j2, prev[2][j2], cc)
                                if jt == 7:
                                    emit_norm(prev[0], prev[1], cc)
                            elif si == 2:
                                # back half: m-half-0 output projection
                                # (ctx(0,0) and ctx(1,0) both normalized)
                                outproj_unit(jt - 8)
                            else:
                                # si == 3: own ctx at 2x in the back half
                                if jt == 8:
                                    cur_cc = [cp.tile([128, 512], FP32, tag="cc",
                                                      name=f"cc_last_{i}")
                                              for i in range(4)]
                                for j2 in ((jt - 8) * 2, (jt - 8) * 2 + 1):
                                    emit_ctx_step(hp, j2, cur_exps[j2], cur_cc)
                        if si == 0 and jt in p0_drib:
                            p0_drib[jt]()
                        elif si == 1 and jt in p1_drib:
                            p1_drib[jt]()
                    prev = (hp, mh, cur_exps)

                # ---- tail: last norm + m-half-1 output projection ----
                emit_norm(1, 1, cur_cc)
                for mt in range(8, 16):
                    outproj_unit(mt)

    nc.compile()
    return nc


def _get_nc():
    global _NC
    if _NC is None:
        _NC = _build_nc()
    return _NC


def _maybe_register_ntff_hook():
    """Optional: register the axon NTFF profile hook so BASS_TRACE=1 yields
    HW exec times. No-op if unavailable (e.g. the grading environment)."""
    if "antenv.axon_hooks" in sys.modules:
        return
    try:
        import types

        if "/root/.axon_site" not in sys.path and os.path.isdir("/root/.axon_site"):
            sys.path.append("/root/.axon_site")
        from trn_agent_boot.trn_boot import _ntff_profile_via_ctypes

        hook = _ntff_profile_via_ctypes("/opt/axon/libaxon_pjrt.so")
        mod = types.ModuleType("antenv.axon_hooks")
        mod.get_axon_ntff_profile_hook = lambda: hook
        mod.set_axon_ntff_profile_hook = lambda h: None
        sys.modules["antenv.axon_hooks"] = mod
    except Exception:
        pass


def kernel(decoder_output, encoder_output, wq, bq, wk, bk, wv, bv, wo, bo):
    from concourse.bass_utils import run_bass_kernel_spmd

    global LAST_RESULTS

    decoder_output = np.asarray(decoder_output, dtype=np.float32)
    encoder_output = np.asarray(encoder_output, dtype=np.float32)
    wq = np.asarray(wq, dtype=np.float32)
    wk = np.asarray(wk, dtype=np.float32)
    wv = np.asarray(wv, dtype=np.float32)
    wo = np.asarray(wo, dtype=np.float32)
    bq = np.asarray(bq, dtype=np.float32)
    bv = np.asarray(bv, dtype=np.float32)
    bo = np.asarray(bo, dtype=np.float32)
    # bk is softmax-invariant (adds a per-query constant to every logit).

    if os.environ.get("BASS_TRACE"):
        _maybe_register_ntff_hook()

    nc = _get_nc()

    xT = {}
    for b in range(B):
        xT[("d", b)] = np.ascontiguousarray(decoder_output[b].T).astype(BF16)
        xT[("e", b)] = np.ascontiguousarray(encoder_output[b].T).astype(BF16)

    def swz(w_slice, t):
        # [t*128, n] -> [128, t*n] so the device DMA reads 4KB/partition
        n = w_slice.shape[1]
        return np.ascontiguousarray(
            w_slice.reshape(t, 128, n).transpose(1, 0, 2).reshape(128, t * n)
        ).astype(BF16)

    in_maps = []
    for c in range(8):
        b, hg = c // 4, c % 4
        sl = slice(hg * NPC, (hg + 1) * NPC)
        in_maps.append({
            "xdT": xT[("d", b)],
            "xeT": xT[("e", b)],
            "wq": swz(wq[:, sl], KT),
            "wk": swz(wk[:, sl], KT),
            "wv": swz(wv[:, sl], KT),
            "wo": swz(wo[sl, :], NT),
            "bq": np.ascontiguousarray(
                bq[sl].reshape(NT, 128).T).astype(np.float32),
        })

    res = run_bass_kernel_spmd(nc, in_maps, core_ids=list(range(8)))
    LAST_RESULTS = res

    correction = (bv @ wo + bo).astype(np.float32)  # probs sum to 1
    out = np.zeros((B, S, D), dtype=np.float32)
    for c in range(8):
        out[c // 4] += np.asarray(res.results[c]["o"]).astype(np.float32)
    out += correction[None, None, :]
    return out


# revision 22
# speedup vs baseline: 1.1393x; 1.0723x over previous
"""Self-contained Trainium2 Bass kernel for 16-head cross-attention MHA.

Problem: B=2, SQ=SK=2048, D=1024, H=16, key_size=64 (fp32 in/out).

Sharding (8 cores): data-parallel over batch (2) x tensor-parallel over
head groups (4 heads per core). Each core computes its 4 heads'
Q/K/V projections (column slices of wq/wk/wv), attention, and a partial
output projection (row slice of wo), uploaded bf16. Host sums the 4
partial outputs per batch and adds the (bv @ wo + bo) correction (probs
sum to 1, so bv contributes exactly bv @ wo; bk cancels in softmax).

Device pipeline per core (bf16 matmuls, fp32 PSUM accumulation), ACT
(ScalarE exp over 4 x 2048 x 2048 scores, ~147us) is the pacer:

  1. Inputs stream in 512-column blocks (wk/wq first) so the first
     score matmul issues ~10us in; the K/Q/V projections for later
     blocks dribble through phase slack.
  2. Score PSUM tiles are laid out per q-chunk as [hh0 | hh1] so one
     exp ACT call drains (and releases) both head-halves together:
     the two K=64 score matmuls of the next tile become ready
     simultaneously and stream CONCURRENTLY in PE row-groups
     (0,0)/(64,0) (tile_position auto-derived from base partitions),
     halving score PE time.
  3. Phases run (hp, mh) = (0,0),(1,0),(0,1),(1,1). ctx for phase i is
     phase-shifted into phase i+1 (PSUM: "ss" 2x[128,1024] q-chunk
     score tiles + "cc" 4x[128,512] chains; V' carries a ones column so
     row 64 accumulates the softmax denominator Z).
  4. ctx(p1) runs 2x-rate in p2's front half, freeing p2's back half
     for the m-half-0 output projection (ctx for m-half 0 is complete
     and normalized by then). ctx(p2) runs 2x in p3's front half,
     ctx(p3) 2x in its back half, leaving only norm + the m-half-1
     output projection for the tail.
  5. Normalization: Z row partition-broadcast via GpSimd, DVE
     reciprocal, DVE multiply into ctx^T bf16.
"""

import os
import sys

for _p in ("/opt/trn_rl_repo", "/root/.axon_site/_ro/trn_rl_repo"):
    if os.path.isdir(_p) and _p not in sys.path:
        sys.path.insert(0, _p)

import numpy as np
import ml_dtypes

BF16 = ml_dtypes.bfloat16

B = 2
S = 2048          # SQ == SK
D = 1024
H = 16
KEY = 64
HPC = 4           # heads per core
NPC = HPC * KEY   # 256 per-core slice of D
KT = D // 128     # 8 contraction tiles for projections
NT = NPC // 128   # 2 head-pair tiles
MC = S // 512     # 4 m-chunks of 512
JT = S // 128     # 16 key tiles
NB = 4            # 512-wide column blocks for streaming/projection

_NC = None
LAST_RESULTS = None  # BassKernelResults of the most recent run (for test.py)


def _build_nc():
    import concourse.tile as tile
    from concourse import bacc, mybir

    FP32 = mybir.dt.float32
    BF = mybir.dt.bfloat16
    AF = mybir.ActivationFunctionType

    nc = bacc.Bacc("TRN2", target_bir_lowering=False, debug=False, num_devices=8)

    # weights arrive host-preswizzled to [128, kt, n] so their DMAs read
    # 4KB-contiguous per partition
    xdT = nc.dram_tensor("xdT", [D, S], BF, kind="ExternalInput").ap()
    xeT = nc.dram_tensor("xeT", [D, S], BF, kind="ExternalInput").ap()
    wq_d = nc.dram_tensor("wq", [128, KT * NPC], BF, kind="ExternalInput").ap()
    wk_d = nc.dram_tensor("wk", [128, KT * NPC], BF, kind="ExternalInput").ap()
    wv_d = nc.dram_tensor("wv", [128, KT * NPC], BF, kind="ExternalInput").ap()
    wo_d = nc.dram_tensor("wo", [128, NT * D], BF, kind="ExternalInput").ap()
    bq_d = nc.dram_tensor("bq", [128, NT], FP32, kind="ExternalInput").ap()
    o_d = nc.dram_tensor("o", [S, D], BF, kind="ExternalOutput").ap()

    xeT_r = xeT.rearrange("(t p) m -> p t m", p=128)
    xdT_r = xdT.rearrange("(t p) m -> p t m", p=128)
    o_r = o_d.rearrange("(g p) n -> p g n", p=128)

    with tile.TileContext(nc) as tc:
        with (
            tc.tile_pool(name="consts", bufs=1) as consts,
            tc.tile_pool(name="acts", bufs=1) as acts,
            tc.tile_pool(name="zp", bufs=2) as zp,
            tc.tile_pool(name="up", bufs=6) as up,
            tc.tile_pool(name="zbp", bufs=2) as zbp,
            tc.tile_pool(name="osb", bufs=2) as osb,
        ):
            # ---- resident weights (wk/wq first: prefix critical path) ----
            wk_sb = consts.tile([128, KT, NPC], BF, tag="wk")
            nc.sync.dma_start(wk_sb[:], wk_d.rearrange("p (t n) -> p t n", t=KT))
            wq_sb = consts.tile([128, KT, NPC], BF, tag="wq")
            nc.sync.dma_start(wq_sb[:], wq_d.rearrange("p (t n) -> p t n", t=KT))
            wv_sb = consts.tile([128, KT, NPC], BF, tag="wv")
            wo_sb = consts.tile([128, NT, D], BF, tag="wo")
            bq_sb = consts.tile([128, NT, 1], FP32, tag="bq")

            # ---- activations kept resident ----
            QT_sb = acts.tile([128, NT, S], BF, tag="QT")    # [head_dim, m]
            KT_sb = acts.tile([128, NT, S], BF, tag="KT")    # [head_dim, j]
            v_sb = acts.tile([128, JT, HPC, KEY + 1], BF, tag="v")  # V' + ones
            ctxT_sb = acts.tile([128, NT, S], BF, tag="ctxT")

            nc.vector.memset(v_sb[:, :, :, KEY:KEY + 1], 1.0)

            # PSUM: "ss" 2x[128,1024] (4 banks) q-chunk score tiles
            #       "cc" 4x[128,512] (4 banks) ctx/proj/out-proj chains
            with (
                tc.tile_pool(name="expp", bufs=34) as expp,
                tc.tile_pool(name="xep", bufs=1) as xep,
                tc.tile_pool(name="xdp", bufs=2) as xdp,
                tc.tile_pool(name="ps", bufs=2, space="PSUM") as ps,
                tc.tile_pool(name="cp", bufs=4, space="PSUM") as cp,
            ):
                # ---- streamed inputs: few, block-sized DMAs that spread
                # across the parallel DMA queues ----
                xeT_sb = xep.tile([128, KT, S], BF, tag="xeT")
                xd_tiles = []
                xb0 = xdp.tile([128, KT, 512], BF, tag="xd", name="xdb_0")
                xd_tiles.append(xb0)
                nc.sync.dma_start(xb0[:], xdT_r[:, :, 0:512])
                xb1 = xdp.tile([128, KT, 512], BF, tag="xd", name="xdb_1")
                xd_tiles.append(xb1)
                nc.sync.dma_start(xb1[:], xdT_r[:, :, 512:1024])
                nc.sync.dma_start(xeT_sb[:, 0:4, 0:512], xeT_r[:, 0:4, 0:512])
                nc.sync.dma_start(xeT_sb[:, 4:8, 0:512], xeT_r[:, 4:8, 0:512])
                nc.sync.dma_start(bq_sb[:], bq_d.rearrange("p (t o) -> p t o", o=1))
                for b in range(1, NB):
                    nc.sync.dma_start(
                        xeT_sb[:, :, b * 512:(b + 1) * 512],
                        xeT_r[:, :, b * 512:(b + 1) * 512],
                    )
                # later-needed weights after the prefix-critical loads
                nc.sync.dma_start(wv_sb[:], wv_d.rearrange("p (t n) -> p t n", t=KT))
                nc.sync.dma_start(wo_sb[:], wo_d.rearrange("p (t n) -> p t n", t=NT))
                # xd blocks 2,3 reuse the 2 xd buffers (WAR-gated until the
                # nt1 Q chains of blocks 0,1 finish in phase-0 dribbles)
                for b in range(2, NB):
                    xb = xdp.tile([128, KT, 512], BF, tag="xd", name=f"xdb_{b}")
                    xd_tiles.append(xb)
                    nc.sync.dma_start(xb[:], xdT_r[:, :, b * 512:(b + 1) * 512])

                # ---- projection work units (chain + evict) ----
                def k_unit(b, nt):
                    ch = cp.tile([128, 512], FP32, tag="cc", name=f"kch_{b}_{nt}")
                    for kt in range(KT):
                        nc.tensor.matmul(
                            ch[:], wk_sb[:, kt, nt * 128:(nt + 1) * 128],
                            xeT_sb[:, kt, b * 512:(b + 1) * 512],
                            start=(kt == 0), stop=(kt == KT - 1),
                        )
                    nc.vector.tensor_copy(KT_sb[:, nt, b * 512:(b + 1) * 512], ch[:])

                def q_unit(b, nt):
                    ch = cp.tile([128, 512], FP32, tag="cc", name=f"qch_{b}_{nt}")
                    for kt in range(KT):
                        nc.tensor.matmul(
                            ch[:], wq_sb[:, kt, nt * 128:(nt + 1) * 128],
                            xd_tiles[b][:, kt, :],
                            start=(kt == 0), stop=(kt == KT - 1),
                        )
                    nc.vector.tensor_scalar_add(
                        QT_sb[:, nt, b * 512:(b + 1) * 512], ch[:], bq_sb[:, nt, :])

                def v_unit(u):
                    # key tiles 2u, 2u+1: V' rows for all 4 heads
                    for dd in range(2):
                        jv = 2 * u + dd
                        ch = cp.tile([128, 512], FP32, tag="cc", name=f"vch_{jv}")
                        for kt in range(KT):
                            nc.tensor.matmul(
                                ch[:, 0:NPC],
                                xeT_sb[:, kt, jv * 128:(jv + 1) * 128],
                                wv_sb[:, kt, :],
                                start=(kt == 0), stop=(kt == KT - 1),
                            )
                        nc.vector.tensor_copy(
                            v_sb[:, jv, :, 0:KEY],
                            ch[:, 0:NPC].rearrange("p (h k) -> p h k", h=HPC),
                        )

                oqueue = {}

                def outproj_unit(mt):
                    # staged in groups of 4 query-tiles -> one 1MB output DMA
                    g = mt // 4
                    if g not in oqueue:
                        oqueue[g] = osb.tile([128, 4, D], BF, tag="ot",
                                             name=f"ot_{g}")
                    ot = oqueue[g]
                    for ec in range(2):
                        po = cp.tile([128, 512], FP32, tag="cc",
                                     name=f"po_{mt}_{ec}")
                        for dt in range(NT):
                            nc.tensor.matmul(
                                po[:],
                                ctxT_sb[:, dt, mt * 128:(mt + 1) * 128],
                                wo_sb[:, dt, ec * 512:(ec + 1) * 512],
                                start=(dt == 0), stop=(dt == NT - 1),
                            )
                        nc.vector.tensor_copy(
                            ot[:, mt % 4, ec * 512:(ec + 1) * 512], po[:])
                    if mt % 4 == 3:
                        nc.sync.dma_start(o_r[:, g * 4:(g + 1) * 4, :], ot[:])

                # ---- prefix: block-0/1 K and Q (nt0) so scores start ASAP
                k_unit(0, 0)
                q_unit(0, 0)
                q_unit(1, 0)
                k_unit(1, 0)

                # dribble schedules: phase -> {jt: emit_fn}
                p0_drib = {
                    0: lambda: k_unit(2, 0), 1: lambda: k_unit(3, 0),
                    2: lambda: k_unit(0, 1), 3: lambda: v_unit(0),
                    4: lambda: k_unit(1, 1), 5: lambda: v_unit(1),
                    6: lambda: k_unit(2, 1), 7: lambda: v_unit(2),
                    8: lambda: k_unit(3, 1), 9: lambda: v_unit(3),
                    10: lambda: q_unit(0, 1), 11: lambda: v_unit(4),
                    12: lambda: q_unit(1, 1), 13: lambda: v_unit(5),
                    14: lambda: v_unit(6), 15: lambda: v_unit(7),
                }
                p1_drib = {
                    0: lambda: q_unit(2, 0), 2: lambda: q_unit(3, 0),
                    4: lambda: q_unit(2, 1), 6: lambda: q_unit(3, 1),
                }

                rows = [0, KEY]

                def emit_scores(si, hp, m0, jt, cur_exps):
                    # two q-chunk tiles, each [hh0 | hh1]; the two K=64
                    # matmuls per chunk stream concurrently (row groups)
                    sst = [ps.tile([128, 1024], FP32, tag="ss",
                                   name=f"ss_{si}_{jt}_{q2}") for q2 in range(2)]
                    for q in range(2):
                        for hh in range(2):
                            row = rows[hh]
                            nc.tensor.matmul(
                                sst[q][:, hh * 512:(hh + 1) * 512],
                                KT_sb[row:row + KEY, hp, jt * 128:(jt + 1) * 128],
                                QT_sb[row:row + KEY, hp,
                                      m0 + q * 512:m0 + (q + 1) * 512],
                                start=True, stop=True,
                            )
                    pair = []
                    for q in range(2):
                        et = expp.tile([128, 1024], BF, tag="exp")
                        nc.scalar.activation(et[:], sst[q][:], AF.Exp, scale=0.125)
                        pair.append(et)
                    cur_exps.append(pair)

                def emit_ctx_step(hp, jt, exp_pair, ccs):
                    for hh in range(2):
                        h = hp * 2 + hh
                        for q in range(2):
                            nc.tensor.matmul(
                                ccs[hh * 2 + q][0:KEY + 1, :],
                                v_sb[:, jt, h, :],
                                exp_pair[q][:, hh * 512:(hh + 1) * 512],
                                start=(jt == 0),
                                stop=(jt == JT - 1),
                            )

                def emit_norm_chunk(hp, mh, hh, q, ccs):
                    m0n = mh * 1024
                    row = rows[hh]
                    c = ccs[hh * 2 + q]
                    u = up.tile([KEY + 1, 512], FP32, tag="u")
                    nc.vector.tensor_copy(u[:], c[0:KEY + 1, :])
                    zraw = zp.tile([1, 512], FP32, tag="z")
                    nc.vector.tensor_copy(zraw[:], u[KEY:KEY + 1, :])
                    zb = zbp.tile([KEY, 512], FP32, tag="zb")
                    nc.gpsimd.partition_broadcast(zb[:], zraw[:])
                    zbr = zbp.tile([KEY, 512], FP32, tag="zbr")
                    nc.vector.reciprocal_approx_fast(zbr[:], zb[:])
                    nc.vector.tensor_mul(
                        ctxT_sb[row:row + KEY, hp,
                                m0n + q * 512:m0n + (q + 1) * 512],
                        u[0:KEY, :],
                        zbr[:],
                    )

                # q0 chunks first so consumers of the first m-chunk unblock
                NORM_ORDER = [(0, 0), (1, 0), (0, 1), (1, 1)]

                def emit_norm(hp, mh, ccs):
                    for hh, q in NORM_ORDER:
                        emit_norm_chunk(hp, mh, hh, q, ccs)

                # ---- phases ----
                order = [(0, 0), (1, 0), (0, 1), (1, 1)]
                prev = None  # (hp, mh, exps)

                for si, (hp, mh) in enumerate(order):
                    m0 = mh * 1024
                    cur_exps = []
                    cc = None
                    cur_cc = None
                    if prev is not None:
                        cc = [cp.tile([128, 512], FP32, tag="cc",
                                      name=f"cc_{si}_{i}") for i in range(4)]
                    for jt in range(JT):
                        emit_scores(si, hp, m0, jt, cur_exps)
                        if si == 1:
                            # ctx(p0), finishing by jt11 so its norm chunks
                            # spread over slots 12-15 and the chain buffers
                            # are free when p2's ctx(p1) starts
                            if jt < 12:
                                emit_ctx_step(prev[0], jt, prev[2][jt], cc)
                                if jt % 3 == 0:
                                    emit_ctx_step(prev[0], 12 + jt // 3,
                                                  prev[2][12 + jt // 3], cc)
                            else:
                                emit_norm_chunk(prev[0], prev[1],
                                                *NORM_ORDER[jt - 12], cc)
                        elif si >= 2:
                            # ctx(prev) at 2x in the front half
                            if jt < 8:
                                for j2 in (jt * 2, jt * 2 + 1):
                                    emit_ctx_step(prev[0], j2, prev[2][j2], cc)
                                if jt == 7:
                                    emit_norm(prev[0], prev[1], cc)
                            elif si == 2:
                                # back half: m-half-0 output projection
                                # (ctx(0,0) and ctx(1,0) both normalized)
                                outproj_unit(jt - 8)
                            else:
                                # si == 3: own ctx at 2x in the back half
                                if jt == 8:
                                    cur_cc = [cp.tile([128, 512], FP32, tag="cc",
                                                      name=f"cc_last_{i}")
                                              for i in range(4)]
                                for j2 in ((jt - 8) * 2, (jt - 8) * 2 + 1):
                                    emit_ctx_step(hp, j2, cur_exps[j2], cur_cc)
                        if si == 0 and jt in p0_drib:
                            p0_drib[jt]()
                        elif si == 1 and jt in p1_drib:
                            p1_drib[jt]()
                    prev = (hp, mh, cur_exps)

                # ---- tail: last norm + m-half-1 output projection ----
                # The dt=0 (head-pair 0) contraction half only needs
                # ctx(0,1), normalized back at p3-jt7 — its matmuls are
                # pre-issued to keep the PE warm while the final norm's
                # gpsimd/DVE chain runs; after each norm chunk lands, only
                # the dt=1 matmuls + eviction remain per query tile.
                emit_norm(1, 1, cur_cc)
                tpo = {}

                def tail_dt0(mt):
                    tpo[mt] = []
                    for ec in range(2):
                        po = cp.tile([128, 512], FP32, tag="cc",
                                     name=f"tpo_{mt}_{ec}")
                        tpo[mt].append(po)
                        nc.tensor.matmul(
                            po[:], ctxT_sb[:, 0, mt * 128:(mt + 1) * 128],
                            wo_sb[:, 0, ec * 512:(ec + 1) * 512],
                            start=True, stop=False,
                        )

                def tail_dt1(mt):
                    g = mt // 4
                    if g not in oqueue:
                        oqueue[g] = osb.tile([128, 4, D], BF, tag="ot",
                                             name=f"ot_{g}")
                    ot = oqueue[g]
                    for ec in range(2):
                        po = tpo[mt][ec]
                        nc.tensor.matmul(
                            po[:], ctxT_sb[:, 1, mt * 128:(mt + 1) * 128],
                            wo_sb[:, 1, ec * 512:(ec + 1) * 512],
                            start=False, stop=True,
                        )
                        # ScalarE is idle in the tail; DVE is on norm duty
                        nc.scalar.copy(ot[:, mt % 4, ec * 512:(ec + 1) * 512],
                                       po[:])
                    if mt % 2 == 1:
                        nc.sync.dma_start(
                            o_r[:, mt - 1:mt + 1, :],
                            ot[:, (mt - 1) % 4:(mt - 1) % 4 + 2, :])

                tail_dt0(8)
                tail_dt0(9)
                for mt in range(8, 16):
                    tail_dt1(mt)
                    if mt + 2 < 16:
                        tail_dt0(mt + 2)

    nc.compile()
    return nc


def _get_nc():
    global _NC
    if _NC is None:
        _NC = _build_nc()
    return _NC


def _maybe_register_ntff_hook():
    """Optional: register the axon NTFF profile hook so BASS_TRACE=1 yields
    HW exec times. No-op if unavailable (e.g. the grading environment)."""
    if "antenv.axon_hooks" in sys.modules:
        return
    try:
        import types

        if "/root/.axon_site" not in sys.path and os.path.isdir("/root/.axon_site"):
            sys.path.append("/root/.axon_site")
        from trn_agent_boot.trn_boot import _ntff_profile_via_ctypes

        hook = _ntff_profile_via_ctypes("/opt/axon/libaxon_pjrt.so")
        mod = types.ModuleType("antenv.axon_hooks")
        mod.get_axon_ntff_profile_hook = lambda: hook
        mod.set_axon_ntff_profile_hook = lambda h: None
        sys.modules["antenv.axon_hooks"] = mod
    except Exception:
        pass


def kernel(decoder_output, encoder_output, wq, bq, wk, bk, wv, bv, wo, bo):
    from concourse.bass_utils import run_bass_kernel_spmd

    global LAST_RESULTS

    decoder_output = np.asarray(decoder_output, dtype=np.float32)
    encoder_output = np.asarray(encoder_output, dtype=np.float32)
    wq = np.asarray(wq, dtype=np.float32)
    wk = np.asarray(wk, dtype=np.float32)
    wv = np.asarray(wv, dtype=np.float32)
    wo = np.asarray(wo, dtype=np.float32)
    bq = np.asarray(bq, dtype=np.float32)
    bv = np.asarray(bv, dtype=np.float32)
    bo = np.asarray(bo, dtype=np.float32)
    # bk is softmax-invariant (adds a per-query constant to every logit).

    if os.environ.get("BASS_TRACE"):
        _maybe_register_ntff_hook()

    nc = _get_nc()

    xT = {}
    for b in range(B):
        xT[("d", b)] = np.ascontiguousarray(decoder_output[b].T).astype(BF16)
        xT[("e", b)] = np.ascontiguousarray(encoder_output[b].T).astype(BF16)

    def swz(w_slice, t):
        # [t*128, n] -> [128, t*n] so the device DMA reads 4KB/partition
        n = w_slice.shape[1]
        return np.ascontiguousarray(
            w_slice.reshape(t, 128, n).transpose(1, 0, 2).reshape(128, t * n)
        ).astype(BF16)

    in_maps = []
    for c in range(8):
        b, hg = c // 4, c % 4
        sl = slice(hg * NPC, (hg + 1) * NPC)
        in_maps.append({
            "xdT": xT[("d", b)],
            "xeT": xT[("e", b)],
            "wq": swz(wq[:, sl], KT),
            "wk": swz(wk[:, sl], KT),
            "wv": swz(wv[:, sl], KT),
            "wo": swz(wo[sl, :], NT),
            "bq": np.ascontiguousarray(
                bq[sl].reshape(NT, 128).T).astype(np.float32),
        })

    res = run_bass_kernel_spmd(nc, in_maps, core_ids=list(range(8)))
    LAST_RESULTS = res

    correction = (bv @ wo + bo).astype(np.float32)  # probs sum to 1
    out = np.zeros((B, S, D), dtype=np.float32)
    for c in range(8):
        out[c // 4] += np.asarray(res.results[c]["o"]).astype(np.float32)
    out += correction[None, None, :]
    return out
